# revision 10
# baseline (speedup 1.0000x reference)
"""Trainium2 Bass kernel for CustomMultiHeadAttention.

Problem: B=4, S=2048, D=1024, H=16 heads (depth 64).
  q = elu(query @ Wq + bq); k = elu(key @ Wk + bk); v = elu(value @ Wv + bv)
  logits = (q_h k_h^T)/8 + mask*-1e9 ; attn = softmax ; ctx = attn v
  out = elu(ctx @ Wo + bo)

Sharding: 8 cores = (batch b in 0..3) x (query-half hf in 0..1).
Each core computes out[b, hf*1024:(hf+1)*1024, :] completely
(K/V projections for the batch are duplicated across the half-pair).
No cross-core communication; host only slices/transposes inputs and
concatenates outputs.

Device-side structure (per core), all matmuls bf16 with fp32 PSUM:
  Phase 1: Q^T [D, S1] and K^T [D, S2] feature-major, V row-major
           [S2, D] augmented with a ones column per head (V').
  Phase 2 (per head): L^T tile = K_h^T^T-matmul, i.e. logits transposed
           [s2-part, s1-free]; E = exp(L/8) * zmask^T (multiplicative
           mask; max-subtraction unnecessary: |logits/8| < 3);
           ctx'^T[65, s1] = V'_h^T @ E accumulated over s2 tiles; row 64
           is the softmax denominator (ones column) for free.
  Phase 3: out = elu(ctx^T.T @ Wo + bo) with per-head K=64 accumulation.
"""

import numpy as np
import ml_dtypes

import concourse.bass as bass
import concourse.tile as tile
from concourse import bacc, mybir
from concourse import bass_utils

BF16 = mybir.dt.bfloat16
F32 = mybir.dt.float32
AF = mybir.ActivationFunctionType
ALU = mybir.AluOpType

B, S, D, H = 4, 2048, 1024, 16
DEP = 64          # head depth
S1 = 1024         # query rows per core
S2 = S            # key rows
P = 128
KT = D // P       # 8 contraction tiles of 128
NT1 = S1 // P     # 8 s1 tiles
NT2 = S2 // P     # 16 s2 tiles
CH = 512          # free-dim chunk (one fp32 PSUM bank)

_nbf16 = ml_dtypes.bfloat16


def _elu_from_psum(nc, scr, psum_ap, bias_pp, out_ap):
    """elu(psum + bias) -> out_ap. bias_pp is a per-partition [P,1] f32 AP
    or None. Uses: ACT relu, DVE stt (x - relu(x) = min(x,0)), ACT exp,
    DVE stt ((e - 1) + relu)."""
    pdim, fdim = psum_ap.shape[0], psum_ap.shape[-1]
    r = scr.tile([P, CH], F32, tag="elu_r", name="elu_r")[:pdim, :fdim]
    m = scr.tile([P, CH], F32, tag="elu_m", name="elu_m")[:pdim, :fdim]
    bias = bias_pp if bias_pp is not None else 0.0
    # r = relu(x + bias)
    nc.scalar.activation(r, psum_ap, AF.Relu, bias=bias, scale=1.0)
    # m = (x + bias) - r = min(x + bias, 0)
    if bias_pp is not None:
        nc.vector.scalar_tensor_tensor(m, psum_ap, bias_pp, r, ALU.add, ALU.subtract)
    else:
        nc.vector.tensor_sub(m, psum_ap, r)
    # m = exp(m)  (in place)
    nc.scalar.activation(m, m, AF.Exp)
    # out = (m - 1) + r
    nc.vector.scalar_tensor_tensor(out_ap, m, -1.0, r, ALU.add, ALU.add)


def _elu_from_sbuf(nc, scr, t_ap, out_ap):
    """elu(t) -> out_ap for an SBUF f32 input (bias already added)."""
    pdim, fdim = t_ap.shape[0], t_ap.shape[-1]
    r = scr.tile([P, CH], F32, tag="elu_r", name="elu_r")[:pdim, :fdim]
    m = scr.tile([P, CH], F32, tag="elu_m", name="elu_m")[:pdim, :fdim]
    nc.scalar.activation(r, t_ap, AF.Relu)
    nc.vector.tensor_sub(m, t_ap, r)
    nc.scalar.activation(m, m, AF.Exp)
    nc.vector.scalar_tensor_tensor(out_ap, m, -1.0, r, ALU.add, ALU.add)


def _build_program():
    nc = bacc.Bacc(
        "TRN2",
        debug=False,
        enable_asserts=False,
        target_bir_lowering=False,
        num_devices=8,
    )

    xqT = nc.dram_tensor("xqT", [D, S1], F32, kind="ExternalInput").ap()
    xkT = nc.dram_tensor("xkT", [D, S2], F32, kind="ExternalInput").ap()
    xvT = nc.dram_tensor("xvT", [D, S2], F32, kind="ExternalInput").ap()
    zmT = nc.dram_tensor("zmT", [S2, S1], BF16, kind="ExternalInput").ap()
    wq = nc.dram_tensor("wq", [D, D], BF16, kind="ExternalInput").ap()
    wk = nc.dram_tensor("wk", [D, D], BF16, kind="ExternalInput").ap()
    wv = nc.dram_tensor("wv", [D, D], BF16, kind="ExternalInput").ap()
    wo = nc.dram_tensor("wo", [DEP, H, D], BF16, kind="ExternalInput").ap()
    bq = nc.dram_tensor("bq", [P, KT], F32, kind="ExternalInput").ap()
    bk = nc.dram_tensor("bk", [P, KT], F32, kind="ExternalInput").ap()
    bv = nc.dram_tensor("bv", [1, D], F32, kind="ExternalInput").ap()
    bo = nc.dram_tensor("bo", [1, D], F32, kind="ExternalInput").ap()
    out = nc.dram_tensor("out", [S1, D], F32, kind="ExternalOutput").ap()

    # DRAM views tiled by 128 partitions
    xqT_t = xqT.rearrange("(t p) s -> p t s", p=P)
    xkT_t = xkT.rearrange("(t p) s -> p t s", p=P)
    xvT_t = xvT.rearrange("(t p) s -> p t s", p=P)
    zmT_t = zmT.rearrange("(t p) s -> p t s", p=P)
    wq_t = wq.rearrange("(t p) f -> p t f", p=P)
    wk_t = wk.rearrange("(t p) f -> p t f", p=P)
    wv_t = wv.rearrange("(t p) f -> p t f", p=P)
    out_t = out.rearrange("(t p) f -> p t f", p=P)

    with tile.TileContext(nc) as tc:
        with (
            tc.tile_pool(name="consts", bufs=1) as consts,
            tc.tile_pool(name="qkv_out", bufs=1) as qkv_out,
        ):
            bq_s = consts.tile([P, KT], F32)
            bk_s = consts.tile([P, KT], F32)
            # bv/bo are free-axis biases: replicate across partitions via
            # broadcast DMA (DVE cannot read partition-step-0 APs).
            bv_s = consts.tile([P, D], F32)
            bo_s = consts.tile([P, D], F32)
            nc.sync.dma_start(bq_s[:], bq)
            nc.sync.dma_start(bk_s[:], bk)
            nc.sync.dma_start(bv_s[:], bv.to_broadcast((P, D)))
            nc.sync.dma_start(bo_s[:], bo.to_broadcast((P, D)))

            qT_s = qkv_out.tile([P, KT, S1], BF16)     # Q^T, head h -> [64*(h%2):, h//2, :]
            kT_s = qkv_out.tile([P, KT, S2], BF16)
            v_s = qkv_out.tile([P, NT2, H, DEP + 1], BF16)  # V' with ones col

            # ones columns of V'
            nc.vector.memset(v_s[:, :, :, DEP : DEP + 1], 1.0)

            # ---------------- Phase 1 ----------------
            with tc.tile_pool(name="p1psum", bufs=4, space="PSUM") as p1psum:
                # --- Q projection ---
                with (
                    tc.tile_pool(name="wq_pool", bufs=1) as wq_pool,
                    tc.tile_pool(name="xq_pool", bufs=1) as xq_pool,
                    tc.tile_pool(name="scr1q", bufs=3) as scr,
                ):
                    wq_s = wq_pool.tile([P, KT, D], BF16)
                    nc.sync.dma_start(wq_s[:], wq_t)
                    # gpsimd (SWDGE) DMA casts f32 -> bf16 in flight
                    xq_s = xq_pool.tile([P, KT, S1], BF16)
                    for k in range(KT):
                        nc.gpsimd.dma_start(xq_s[:, k, :], xqT_t[:, k, :])
                    # Q^T = Wq^T @ Xq^T  (lhsT = Wq block, rhs = Xq^T)
                    for ft in range(KT):
                        for c in range(S1 // CH):
                            ps = p1psum.tile([P, CH], F32, tag="p1", name="p1")
                            for k in range(KT):
                                nc.tensor.matmul(
                                    ps[:],
                                    wq_s[:, k, ft * P : (ft + 1) * P],
                                    xq_s[:, k, c * CH : (c + 1) * CH],
                                    start=(k == 0),
                                    stop=(k == KT - 1),
                                )
                            _elu_from_psum(
                                nc, scr, ps[:], bq_s[:, ft : ft + 1],
                                qT_s[:, ft, c * CH : (c + 1) * CH],
                            )
                # --- K projection ---
                with (
                    tc.tile_pool(name="wk_pool", bufs=1) as wk_pool,
                    tc.tile_pool(name="xk_pool", bufs=1) as xk_pool,
                    tc.tile_pool(name="scr1k", bufs=3) as scr,
                ):
                    wk_s = wk_pool.tile([P, KT, D], BF16)
                    nc.sync.dma_start(wk_s[:], wk_t)
                    xk_s = xk_pool.tile([P, KT, S2], BF16)
                    for k in range(KT):
                        nc.gpsimd.dma_start(xk_s[:, k, :], xkT_t[:, k, :])
                    for ft in range(KT):
                        for c in range(S2 // CH):
                            ps = p1psum.tile([P, CH], F32, tag="p1", name="p1")
                            for k in range(KT):
                                nc.tensor.matmul(
                                    ps[:],
                                    wk_s[:, k, ft * P : (ft + 1) * P],
                                    xk_s[:, k, c * CH : (c + 1) * CH],
                                    start=(k == 0),
                                    stop=(k == KT - 1),
                                )
                            _elu_from_psum(
                                nc, scr, ps[:], bk_s[:, ft : ft + 1],
                                kT_s[:, ft, c * CH : (c + 1) * CH],
                            )
                # --- V projection (row-major, into V' with ones col) ---
                with (
                    tc.tile_pool(name="wv_pool", bufs=1) as wv_pool,
                    tc.tile_pool(name="xv_pool", bufs=1) as xv_pool,
                    tc.tile_pool(name="scr1v", bufs=3) as scr,
                ):
                    wv_s = wv_pool.tile([P, KT, D], BF16)
                    nc.sync.dma_start(wv_s[:], wv_t)
                    xv_s = xv_pool.tile([P, KT, S2], BF16)
                    for k in range(KT):
                        nc.gpsimd.dma_start(xv_s[:, k, :], xvT_t[:, k, :])
                    for t2 in range(NT2):
                        for c in range(D // CH):
                            ps = p1psum.tile([P, CH], F32, tag="p1", name="p1")
                            for k in range(KT):
                                nc.tensor.matmul(
                                    ps[:],
                                    xv_s[:, k, t2 * P : (t2 + 1) * P],
                                    wv_s[:, k, c * CH : (c + 1) * CH],
                                    start=(k == 0),
                                    stop=(k == KT - 1),
                                )
                            tt = scr.tile([P, CH], F32, tag="vtmp", name="vtmp")
                            nc.vector.tensor_add(
                                tt[:], ps[:],
                                bv_s[:, c * CH : (c + 1) * CH],
                            )
                            # chunk c covers heads 8c..8c+7
                            dst = v_s[:, t2, 8 * c : 8 * (c + 1), 0:DEP]
                            _elu_from_sbuf(nc, scr, tt[:], dst)

            # ---------------- Phase 2 ----------------
            with tc.tile_pool(name="ctx", bufs=1) as ctxp:
                ctxT_s = ctxp.tile([DEP, H, S1], BF16)
                with (
                    tc.tile_pool(name="zm", bufs=1) as zm,
                    tc.tile_pool(name="epool", bufs=6) as epool,
                    tc.tile_pool(name="scr2", bufs=4) as scr,
                    tc.tile_pool(name="p2psum", bufs=4, space="PSUM") as p2psum,
                    tc.tile_pool(name="ctxpsum", bufs=3, space="PSUM") as ctxpsum,
                ):
                    zm_s = zm.tile([P, NT2, S1], BF16)
                    for t2 in range(NT2):
                        nc.sync.dma_start(zm_s[:, t2, :], zmT_t[:, t2, :])

                    for hp in range(H // 2):      # head pairs for PE row-packing
                        heads = (2 * hp, 2 * hp + 1)
                        for c in range(S1 // CH):
                            pcs = [
                                ctxpsum.tile(
                                    [DEP + 1, CH], F32, tag="ctxps", name="ctxps"
                                )
                                for _ in heads
                            ]
                            for t2 in range(NT2):
                                for i, h in enumerate(heads):
                                    base = DEP * (h % 2)
                                    lps = p2psum.tile([P, CH], F32, tag="lps", name="lps")
                                    # L^T tile: lhsT = K^T_h s2-block, rhs = Q^T_h
                                    nc.tensor.matmul(
                                        lps[:],
                                        kT_s[base : base + DEP, h // 2,
                                             t2 * P : (t2 + 1) * P],
                                        qT_s[base : base + DEP, h // 2,
                                             c * CH : (c + 1) * CH],
                                        start=True,
                                        stop=True,
                                    )
                                    e_s = epool.tile([P, CH], BF16, tag="e", name="e_s")
                                    nc.scalar.activation(
                                        e_s[:], lps[:], AF.Exp, scale=0.125
                                    )
                                    nc.vector.tensor_mul(
                                        e_s[:], e_s[:],
                                        zm_s[:, t2, c * CH : (c + 1) * CH],
                                    )
                                    nc.tensor.matmul(
                                        pcs[i][:],
                                        v_s[:, t2, h, :],
                                        e_s[:],
                                        start=(t2 == 0),
                                        stop=(t2 == NT2 - 1),
                                    )
                            for i, h in enumerate(heads):
                                rec = scr.tile([1, CH], F32, tag="rec", name="rec")
                                nc.vector.reciprocal(rec[:], pcs[i][DEP : DEP + 1, :])
                                rrep = scr.tile([DEP, CH], F32, tag="rrep", name="rrep")
                                nc.gpsimd.partition_broadcast(rrep[:], rec[:])
                                nc.vector.tensor_mul(
                                    ctxT_s[:, h, c * CH : (c + 1) * CH],
                                    pcs[i][0:DEP, :],
                                    rrep[:],
                                )

                # ---------------- Phase 3 ----------------
                with (
                    tc.tile_pool(name="wo_pool", bufs=1) as wo_pool,
                    tc.tile_pool(name="ostage", bufs=3) as ostage,
                    tc.tile_pool(name="scr3", bufs=3) as scr,
                    tc.tile_pool(name="p3psum", bufs=2, space="PSUM") as p3psum,
                ):
                    wo_s = wo_pool.tile([DEP, H, D], BF16)
                    nc.sync.dma_start(wo_s[:], wo)

                    for t1 in range(NT1):
                        for c in range(D // CH):
                            ps = p3psum.tile([P, CH], F32, tag="p3", name="p3")
                            for h in range(H):
                                nc.tensor.matmul(
                                    ps[:],
                                    ctxT_s[:, h, t1 * P : (t1 + 1) * P],
                                    wo_s[:, h, c * CH : (c + 1) * CH],
                                    start=(h == 0),
                                    stop=(h == H - 1),
                                )
                            tt = scr.tile([P, CH], F32, tag="otmp", name="otmp")
                            nc.vector.tensor_add(
                                tt[:], ps[:],
                                bo_s[:, c * CH : (c + 1) * CH],
                            )
                            ot = ostage.tile([P, CH], F32, tag="ost", name="ost")
                            _elu_from_sbuf(nc, scr, tt[:], ot[:])
                            nc.sync.dma_start(
                                out_t[:, t1, c * CH : (c + 1) * CH], ot[:]
                            )

    nc.compile()
    return nc


_CACHE = {}


def _get_nc():
    if "nc" not in _CACHE:
        _CACHE["nc"] = _build_program()
    return _CACHE["nc"]


def _prep_inputs(value, key, query, mask, Wq, bq, Wk, bk, Wv, bv, Wo, bo):
    f32 = np.float32
    wq16 = np.ascontiguousarray(Wq, f32).astype(_nbf16)
    wk16 = np.ascontiguousarray(Wk, f32).astype(_nbf16)
    wv16 = np.ascontiguousarray(Wv, f32).astype(_nbf16)
    # Wo [D, D] -> [DEP, H, D] with wo[p, h, f] = Wo[64h+p, f]
    wo16 = np.ascontiguousarray(
        np.asarray(Wo, f32).reshape(H, DEP, D).transpose(1, 0, 2)
    ).astype(_nbf16)
    bq_r = np.ascontiguousarray(np.asarray(bq, f32).reshape(KT, P).T)
    bk_r = np.ascontiguousarray(np.asarray(bk, f32).reshape(KT, P).T)
    bv_r = np.asarray(bv, f32).reshape(1, D)
    bo_r = np.asarray(bo, f32).reshape(1, D)

    in_maps = []
    for b in range(B):
        xkT = np.ascontiguousarray(np.asarray(key[b], f32).T)
        xvT = np.ascontiguousarray(np.asarray(value[b], f32).T)
        zT = np.ascontiguousarray(
            (1 - np.asarray(mask[b, 0])).T.astype(_nbf16)
        )  # [S2, S1_full]
        qT = np.ascontiguousarray(np.asarray(query[b], f32).T)  # [D, S]
        for hf in range(2):
            sl = slice(hf * S1, (hf + 1) * S1)
            in_maps.append(
                dict(
                    xqT=np.ascontiguousarray(qT[:, sl]),
                    xkT=xkT,
                    xvT=xvT,
                    zmT=np.ascontiguousarray(zT[:, sl]),
                    wq=wq16, wk=wk16, wv=wv16, wo=wo16,
                    bq=bq_r, bk=bk_r, bv=bv_r, bo=bo_r,
                )
            )
    return in_maps


def kernel(value, key, query, mask, Wq, bq, Wk, bk, Wv, bv, Wo, bo, **run_kwargs):
    nc = _get_nc()
    in_maps = _prep_inputs(
        value, key, query, mask, Wq, bq, Wk, bk, Wv, bv, Wo, bo
    )
    res = bass_utils.run_bass_kernel_spmd(
        nc, in_maps, core_ids=list(range(8)), **run_kwargs
    )
    out = np.empty((B, S, D), np.float32)
    for c in range(8):
        b, hf = c // 2, c % 2
        out[b, hf * S1 : (hf + 1) * S1, :] = res.results[c]["out"]
    if run_kwargs:
        _CACHE["last_results"] = res
    return out


# revision 12
# speedup vs baseline: 1.2486x; 1.2486x over previous
"""Trainium2 Bass kernel for CustomMultiHeadAttention.

Problem: B=4, S=2048, D=1024, H=16 heads (depth 64).
  q = elu(query @ Wq + bq); k = elu(key @ Wk + bk); v = elu(value @ Wv + bv)
  logits = (q_h k_h^T)/8 + mask*-1e9 ; attn = softmax ; ctx = attn v
  out = elu(ctx @ Wo + bo)

Sharding: 8 cores = (batch b in 0..3) x (query-half hf in 0..1).
Each core computes out[b, hf*1024:(hf+1)*1024, :] completely
(K/V projections for the batch are duplicated across the half-pair).
No cross-core communication; host only slices/transposes inputs and
concatenates outputs.

Device-side structure (per core), all matmuls bf16 with fp32 PSUM:
  Phase 1: Q^T [D, S1] and K^T [D, S2] feature-major, V row-major
           [S2, D] augmented with a ones column per head (V').
  Phase 2 (per head): L^T tile = K_h^T^T-matmul, i.e. logits transposed
           [s2-part, s1-free]; E = exp(L/8) * zmask^T (multiplicative
           mask; max-subtraction unnecessary: |logits/8| < 3);
           ctx'^T[65, s1] = V'_h^T @ E accumulated over s2 tiles; row 64
           is the softmax denominator (ones column) for free.
  Phase 3: out = elu(ctx^T.T @ Wo + bo) with per-head K=64 accumulation.
"""

import numpy as np
import ml_dtypes

import concourse.bass as bass
import concourse.tile as tile
from concourse import bacc, mybir
from concourse import bass_utils

BF16 = mybir.dt.bfloat16
F32 = mybir.dt.float32
AF = mybir.ActivationFunctionType
ALU = mybir.AluOpType

B, S, D, H = 4, 2048, 1024, 16
DEP = 64          # head depth
S1 = 1024         # query rows per core
S2 = S            # key rows
P = 128
KT = D // P       # 8 contraction tiles of 128
NT1 = S1 // P     # 8 s1 tiles
NT2 = S2 // P     # 16 s2 tiles
CH = 512          # free-dim chunk (one fp32 PSUM bank)

_nbf16 = ml_dtypes.bfloat16


def _elu_from_psum(nc, scr, psum_ap, bias_pp, out_ap):
    """elu(psum + bias) -> out_ap. bias_pp is a per-partition [P,1] f32 AP
    or None. Uses: ACT relu, DVE stt (x - relu(x) = min(x,0)), ACT exp,
    DVE stt ((e - 1) + relu)."""
    pdim, fdim = psum_ap.shape[0], psum_ap.shape[-1]
    r = scr.tile([P, CH], F32, tag="elu_r", name="elu_r")[:pdim, :fdim]
    m = scr.tile([P, CH], F32, tag="elu_m", name="elu_m")[:pdim, :fdim]
    bias = bias_pp if bias_pp is not None else 0.0
    # r = relu(x + bias)
    nc.scalar.activation(r, psum_ap, AF.Relu, bias=bias, scale=1.0)
    # m = (x + bias) - r = min(x + bias, 0)
    if bias_pp is not None:
        nc.vector.scalar_tensor_tensor(m, psum_ap, bias_pp, r, ALU.add, ALU.subtract)
    else:
        nc.vector.tensor_sub(m, psum_ap, r)
    # m = exp(m)  (in place)
    nc.scalar.activation(m, m, AF.Exp)
    # out = (m - 1) + r
    nc.vector.scalar_tensor_tensor(out_ap, m, -1.0, r, ALU.add, ALU.add)


def _elu_from_sbuf(nc, scr, t_ap, out_ap):
    """elu(t) -> out_ap for an SBUF f32 input (bias already added)."""
    pdim, fdim = t_ap.shape[0], t_ap.shape[-1]
    r = scr.tile([P, CH], F32, tag="elu_r", name="elu_r")[:pdim, :fdim]
    m = scr.tile([P, CH], F32, tag="elu_m", name="elu_m")[:pdim, :fdim]
    nc.scalar.activation(r, t_ap, AF.Relu)
    nc.vector.tensor_sub(m, t_ap, r)
    nc.scalar.activation(m, m, AF.Exp)
    nc.vector.scalar_tensor_tensor(out_ap, m, -1.0, r, ALU.add, ALU.add)


def _build_program():
    nc = bacc.Bacc(
        "TRN2",
        debug=False,
        enable_asserts=False,
        target_bir_lowering=False,
        num_devices=8,
    )

    xqT = nc.dram_tensor("xqT", [D, S1], F32, kind="ExternalInput").ap()
    xkT = nc.dram_tensor("xkT", [D, S2], F32, kind="ExternalInput").ap()
    xvT = nc.dram_tensor("xvT", [D, S2], F32, kind="ExternalInput").ap()
    zmT = nc.dram_tensor("zmT", [S2, S1], BF16, kind="ExternalInput").ap()
    wq = nc.dram_tensor("wq", [D, D], BF16, kind="ExternalInput").ap()
    wk = nc.dram_tensor("wk", [D, D], BF16, kind="ExternalInput").ap()
    wv = nc.dram_tensor("wv", [D, D], BF16, kind="ExternalInput").ap()
    wo = nc.dram_tensor("wo", [D, D], BF16, kind="ExternalInput").ap()
    bq = nc.dram_tensor("bq", [P, KT], F32, kind="ExternalInput").ap()
    bk = nc.dram_tensor("bk", [P, KT], F32, kind="ExternalInput").ap()
    bv = nc.dram_tensor("bv", [1, D], F32, kind="ExternalInput").ap()
    bo = nc.dram_tensor("bo", [1, D], F32, kind="ExternalInput").ap()
    out = nc.dram_tensor("out", [S1, D], F32, kind="ExternalOutput").ap()

    # DRAM views tiled by 128 partitions
    xqT_t = xqT.rearrange("(t p) s -> p t s", p=P)
    xkT_t = xkT.rearrange("(t p) s -> p t s", p=P)
    xvT_t = xvT.rearrange("(t p) s -> p t s", p=P)
    zmT_t = zmT.rearrange("(t p) s -> p t s", p=P)
    wq_t = wq.rearrange("(t p) f -> p t f", p=P)
    wk_t = wk.rearrange("(t p) f -> p t f", p=P)
    wv_t = wv.rearrange("(t p) f -> p t f", p=P)
    wo_t = wo.rearrange("(t p) f -> p t f", p=P)
    out_t = out.rearrange("(t p) f -> p t f", p=P)

    with tile.TileContext(nc) as tc:
        with (
            tc.tile_pool(name="consts", bufs=1) as consts,
            tc.tile_pool(name="qkv_out", bufs=1) as qkv_out,
        ):
            bq_s = consts.tile([P, KT], F32)
            bk_s = consts.tile([P, KT], F32)
            # bv/bo are free-axis biases: replicate across partitions via
            # broadcast DMA (DVE cannot read partition-step-0 APs).
            bv_s = consts.tile([P, D], F32)
            bo_s = consts.tile([P, D], F32)
            nc.sync.dma_start(bq_s[:], bq)
            nc.sync.dma_start(bk_s[:], bk)
            nc.sync.dma_start(bv_s[:], bv.to_broadcast((P, D)))
            nc.sync.dma_start(bo_s[:], bo.to_broadcast((P, D)))

            qT_s = qkv_out.tile([P, KT, S1], BF16)     # Q^T, head h -> [64*(h%2):, h//2, :]
            kT_s = qkv_out.tile([P, KT, S2], BF16)
            v_s = qkv_out.tile([P, NT2, H, DEP + 1], BF16)  # V' with ones col

            # ones columns of V'
            nc.vector.memset(v_s[:, :, :, DEP : DEP + 1], 1.0)

            # ---------------- Phase 1 ----------------
            with tc.tile_pool(name="p1psum", bufs=4, space="PSUM") as p1psum:
                # --- Q projection ---
                with (
                    tc.tile_pool(name="wq_pool", bufs=1) as wq_pool,
                    tc.tile_pool(name="xq_pool", bufs=1) as xq_pool,
                    tc.tile_pool(name="scr1q", bufs=3) as scr,
                ):
                    wq_s = wq_pool.tile([P, KT, D], BF16)
                    nc.sync.dma_start(wq_s[:], wq_t)
                    # gpsimd (SWDGE) DMA casts f32 -> bf16 in flight
                    xq_s = xq_pool.tile([P, KT, S1], BF16)
                    for k in range(KT):
                        nc.gpsimd.dma_start(xq_s[:, k, :], xqT_t[:, k, :])
                    # Q^T = Wq^T @ Xq^T  (lhsT = Wq block, rhs = Xq^T)
                    for ft in range(KT):
                        for c in range(S1 // CH):
                            ps = p1psum.tile([P, CH], F32, tag="p1", name="p1")
                            for k in range(KT):
                                nc.tensor.matmul(
                                    ps[:],
                                    wq_s[:, k, ft * P : (ft + 1) * P],
                                    xq_s[:, k, c * CH : (c + 1) * CH],
                                    start=(k == 0),
                                    stop=(k == KT - 1),
                                )
                            _elu_from_psum(
                                nc, scr, ps[:], bq_s[:, ft : ft + 1],
                                qT_s[:, ft, c * CH : (c + 1) * CH],
                            )
                # --- K projection ---
                with (
                    tc.tile_pool(name="wk_pool", bufs=1) as wk_pool,
                    tc.tile_pool(name="xk_pool", bufs=1) as xk_pool,
                    tc.tile_pool(name="scr1k", bufs=3) as scr,
                ):
                    wk_s = wk_pool.tile([P, KT, D], BF16)
                    nc.sync.dma_start(wk_s[:], wk_t)
                    xk_s = xk_pool.tile([P, KT, S2], BF16)
                    for k in range(KT):
                        nc.gpsimd.dma_start(xk_s[:, k, :], xkT_t[:, k, :])
                    for ft in range(KT):
                        for c in range(S2 // CH):
                            ps = p1psum.tile([P, CH], F32, tag="p1", name="p1")
                            for k in range(KT):
                                nc.tensor.matmul(
                                    ps[:],
                                    wk_s[:, k, ft * P : (ft + 1) * P],
                                    xk_s[:, k, c * CH : (c + 1) * CH],
                                    start=(k == 0),
                                    stop=(k == KT - 1),
                                )
                            _elu_from_psum(
                                nc, scr, ps[:], bk_s[:, ft : ft + 1],
                                kT_s[:, ft, c * CH : (c + 1) * CH],
                            )
                # --- V projection (row-major, into V' with ones col) ---
                with (
                    tc.tile_pool(name="wv_pool", bufs=1) as wv_pool,
                    tc.tile_pool(name="xv_pool", bufs=1) as xv_pool,
                    tc.tile_pool(name="scr1v", bufs=3) as scr,
                ):
                    wv_s = wv_pool.tile([P, KT, D], BF16)
                    nc.sync.dma_start(wv_s[:], wv_t)
                    xv_s = xv_pool.tile([P, KT, S2], BF16)
                    for k in range(KT):
                        nc.gpsimd.dma_start(xv_s[:, k, :], xvT_t[:, k, :])
                    for t2 in range(NT2):
                        for c in range(D // CH):
                            ps = p1psum.tile([P, CH], F32, tag="p1", name="p1")
                            for k in range(KT):
                                nc.tensor.matmul(
                                    ps[:],
                                    xv_s[:, k, t2 * P : (t2 + 1) * P],
                                    wv_s[:, k, c * CH : (c + 1) * CH],
                                    start=(k == 0),
                                    stop=(k == KT - 1),
                                )
                            tt = scr.tile([P, CH], F32, tag="vtmp", name="vtmp")
                            nc.vector.tensor_add(
                                tt[:], ps[:],
                                bv_s[:, c * CH : (c + 1) * CH],
                            )
                            # chunk c covers heads 8c..8c+7
                            dst = v_s[:, t2, 8 * c : 8 * (c + 1), 0:DEP]
                            _elu_from_sbuf(nc, scr, tt[:], dst)

            # ---------------- Phase 2 ----------------
            with tc.tile_pool(name="ctx", bufs=1) as ctxp:
                # ctxT packed: head h -> partitions 64*(h%2).., k-tile h//2
                ctxT_s = ctxp.tile([P, H // 2, S1], BF16)
                with (
                    tc.tile_pool(name="zm", bufs=1) as zm,
                    tc.tile_pool(name="epool", bufs=3) as epool,
                    tc.tile_pool(name="scr2", bufs=4) as scr,
                    tc.tile_pool(name="p2psum", bufs=2, space="PSUM") as p2psum,
                    tc.tile_pool(name="ctxpsum", bufs=4, space="PSUM") as ctxpsum,
                ):
                    zm_s = zm.tile([P, NT2, S1], BF16)
                    for t2 in range(NT2):
                        nc.sync.dma_start(zm_s[:, t2, :], zmT_t[:, t2, :])

                    NCH2 = S1 // CH  # 2 chunks of 512
                    for h in range(H):
                        base = DEP * (h % 2)
                        pcs = [
                            ctxpsum.tile([DEP + 1, CH], F32, tag="ctxps", name="ctxps")
                            for _ in range(NCH2)
                        ]
                        for t2 in range(NT2):
                            # L^T for the whole s1 range in one 2-bank psum
                            lps = p2psum.tile([P, S1], F32, tag="lps", name="lps")
                            for c in range(NCH2):
                                nc.tensor.matmul(
                                    lps[:, c * CH : (c + 1) * CH],
                                    kT_s[base : base + DEP, h // 2,
                                         t2 * P : (t2 + 1) * P],
                                    qT_s[base : base + DEP, h // 2,
                                         c * CH : (c + 1) * CH],
                                    start=True,
                                    stop=True,
                                )
                            e_s = epool.tile([P, S1], BF16, tag="e", name="e_s")
                            nc.scalar.activation(e_s[:], lps[:], AF.Exp, scale=0.125)
                            nc.vector.tensor_mul(e_s[:], e_s[:], zm_s[:, t2, :])
                            for c in range(NCH2):
                                nc.tensor.matmul(
                                    pcs[c][:],
                                    v_s[:, t2, h, :],
                                    e_s[:, c * CH : (c + 1) * CH],
                                    start=(t2 == 0),
                                    stop=(t2 == NT2 - 1),
                                )
                        for c in range(NCH2):
                            rec = scr.tile([1, CH], F32, tag="rec", name="rec")
                            nc.vector.reciprocal(rec[:], pcs[c][DEP : DEP + 1, :])
                            rrep = scr.tile([DEP, CH], F32, tag="rrep", name="rrep")
                            nc.gpsimd.partition_broadcast(rrep[:], rec[:])
                            if h % 2 == 0:
                                nc.vector.tensor_mul(
                                    ctxT_s[0:DEP, h // 2, c * CH : (c + 1) * CH],
                                    pcs[c][0:DEP, :],
                                    rrep[:],
                                )
                            else:
                                # engines cannot partition-shift; stage at
                                # base 0 and DMA into partitions 64..127
                                ctmp = scr.tile([DEP, CH], BF16, tag="ctmp", name="ctmp")
                                nc.vector.tensor_mul(ctmp[:], pcs[c][0:DEP, :], rrep[:])
                                nc.sync.dma_start(
                                    ctxT_s[DEP:P, h // 2, c * CH : (c + 1) * CH],
                                    ctmp[:],
                                )

                # ---------------- Phase 3 ----------------
                with (
                    tc.tile_pool(name="wo_pool", bufs=1) as wo_pool,
                    tc.tile_pool(name="ostage", bufs=3) as ostage,
                    tc.tile_pool(name="scr3", bufs=3) as scr,
                    tc.tile_pool(name="p3psum", bufs=2, space="PSUM") as p3psum,
                ):
                    wo_s = wo_pool.tile([P, KT, D], BF16)
                    nc.sync.dma_start(wo_s[:], wo_t)

                    for t1 in range(NT1):
                        for c in range(D // CH):
                            ps = p3psum.tile([P, CH], F32, tag="p3", name="p3")
                            for kt in range(KT):
                                nc.tensor.matmul(
                                    ps[:],
                                    ctxT_s[:, kt, t1 * P : (t1 + 1) * P],
                                    wo_s[:, kt, c * CH : (c + 1) * CH],
                                    start=(kt == 0),
                                    stop=(kt == KT - 1),
                                )
                            tt = scr.tile([P, CH], F32, tag="otmp", name="otmp")
                            nc.vector.tensor_add(
                                tt[:], ps[:],
                                bo_s[:, c * CH : (c + 1) * CH],
                            )
                            ot = ostage.tile([P, CH], F32, tag="ost", name="ost")
                            _elu_from_sbuf(nc, scr, tt[:], ot[:])
                            nc.sync.dma_start(
                                out_t[:, t1, c * CH : (c + 1) * CH], ot[:]
                            )

    nc.compile()
    return nc


_CACHE = {}


def _get_nc():
    if "nc" not in _CACHE:
        _CACHE["nc"] = _build_program()
    return _CACHE["nc"]


def _prep_inputs(value, key, query, mask, Wq, bq, Wk, bk, Wv, bv, Wo, bo):
    f32 = np.float32
    wq16 = np.ascontiguousarray(Wq, f32).astype(_nbf16)
    wk16 = np.ascontiguousarray(Wk, f32).astype(_nbf16)
    wv16 = np.ascontiguousarray(Wv, f32).astype(_nbf16)
    wo16 = np.ascontiguousarray(Wo, f32).astype(_nbf16)
    bq_r = np.ascontiguousarray(np.asarray(bq, f32).reshape(KT, P).T)
    bk_r = np.ascontiguousarray(np.asarray(bk, f32).reshape(KT, P).T)
    bv_r = np.asarray(bv, f32).reshape(1, D)
    bo_r = np.asarray(bo, f32).reshape(1, D)

    in_maps = []
    for b in range(B):
        xkT = np.ascontiguousarray(np.asarray(key[b], f32).T)
        xvT = np.ascontiguousarray(np.asarray(value[b], f32).T)
        zT = np.ascontiguousarray(
            (1 - np.asarray(mask[b, 0])).T.astype(_nbf16)
        )  # [S2, S1_full]
        qT = np.ascontiguousarray(np.asarray(query[b], f32).T)  # [D, S]
        for hf in range(2):
            sl = slice(hf * S1, (hf + 1) * S1)
            in_maps.append(
                dict(
                    xqT=np.ascontiguousarray(qT[:, sl]),
                    xkT=xkT,
                    xvT=xvT,
                    zmT=np.ascontiguousarray(zT[:, sl]),
                    wq=wq16, wk=wk16, wv=wv16, wo=wo16,
                    bq=bq_r, bk=bk_r, bv=bv_r, bo=bo_r,
                )
            )
    return in_maps


def kernel(value, key, query, mask, Wq, bq, Wk, bk, Wv, bv, Wo, bo, **run_kwargs):
    nc = _get_nc()
    in_maps = _prep_inputs(
        value, key, query, mask, Wq, bq, Wk, bk, Wv, bv, Wo, bo
    )
    res = bass_utils.run_bass_kernel_spmd(
        nc, in_maps, core_ids=list(range(8)), **run_kwargs
    )
    out = np.empty((B, S, D), np.float32)
    for c in range(8):
        b, hf = c // 2, c % 2
        out[b, hf * S1 : (hf + 1) * S1, :] = res.results[c]["out"]
    if run_kwargs:
        _CACHE["last_results"] = res
    return out


# revision 13
# speedup vs baseline: 1.2550x; 1.0052x over previous
"""Trainium2 Bass kernel for CustomMultiHeadAttention.

Problem: B=4, S=2048, D=1024, H=16 heads (depth 64).
  q = elu(query @ Wq + bq); k = elu(key @ Wk + bk); v = elu(value @ Wv + bv)
  logits = (q_h k_h^T)/8 + mask*-1e9 ; attn = softmax ; ctx = attn v
  out = elu(ctx @ Wo + bo)

Sharding: 8 cores = (batch b in 0..3) x (query-half hf in 0..1).
Each core computes out[b, hf*1024:(hf+1)*1024, :] completely
(K/V projections for the batch are duplicated across the half-pair).
No cross-core communication; host only slices/transposes inputs and
concatenates outputs.

Device-side structure (per core), all matmuls bf16 with fp32 PSUM:
  Phase 1: Q^T [D, S1] and K^T [D, S2] feature-major, V row-major
           [S2, D] augmented with a ones column per head (V').
  Phase 2 (per head): L^T tile = K_h^T^T-matmul, i.e. logits transposed
           [s2-part, s1-free]; E = exp(L/8) * zmask^T (multiplicative
           mask; max-subtraction unnecessary: |logits/8| < 3);
           ctx'^T[65, s1] = V'_h^T @ E accumulated over s2 tiles; row 64
           is the softmax denominator (ones column) for free.
  Phase 3: out = elu(ctx^T.T @ Wo + bo) with per-head K=64 accumulation.
"""

import numpy as np
import ml_dtypes

import concourse.bass as bass
import concourse.tile as tile
from concourse import bacc, mybir
from concourse import bass_utils

BF16 = mybir.dt.bfloat16
F32 = mybir.dt.float32
AF = mybir.ActivationFunctionType
ALU = mybir.AluOpType

B, S, D, H = 4, 2048, 1024, 16
DEP = 64          # head depth
S1 = 1024         # query rows per core
S2 = S            # key rows
P = 128
KT = D // P       # 8 contraction tiles of 128
NT1 = S1 // P     # 8 s1 tiles
NT2 = S2 // P     # 16 s2 tiles
CH = 512          # free-dim chunk (one fp32 PSUM bank)

_nbf16 = ml_dtypes.bfloat16


def _elu_from_psum(nc, scr, psum_ap, bias_pp, out_ap):
    """elu(psum + bias) -> out_ap. bias_pp is a per-partition [P,1] f32 AP
    or None. Uses: ACT relu, DVE stt (x - relu(x) = min(x,0)), ACT exp,
    DVE stt ((e - 1) + relu)."""
    pdim, fdim = psum_ap.shape[0], psum_ap.shape[-1]
    r = scr.tile([P, CH], F32, tag="elu_r", name="elu_r")[:pdim, :fdim]
    m = scr.tile([P, CH], F32, tag="elu_m", name="elu_m")[:pdim, :fdim]
    bias = bias_pp if bias_pp is not None else 0.0
    # r = relu(x + bias)
    nc.scalar.activation(r, psum_ap, AF.Relu, bias=bias, scale=1.0)
    # m = (x + bias) - r = min(x + bias, 0)
    if bias_pp is not None:
        nc.vector.scalar_tensor_tensor(m, psum_ap, bias_pp, r, ALU.add, ALU.subtract)
    else:
        nc.vector.tensor_sub(m, psum_ap, r)
    # m = exp(m)  (in place)
    nc.scalar.activation(m, m, AF.Exp)
    # out = (m - 1) + r
    nc.vector.scalar_tensor_tensor(out_ap, m, -1.0, r, ALU.add, ALU.add)


def _elu_from_sbuf(nc, scr, t_ap, out_ap):
    """elu(t) -> out_ap for an SBUF f32 input (bias already added)."""
    pdim, fdim = t_ap.shape[0], t_ap.shape[-1]
    r = scr.tile([P, CH], F32, tag="elu_r", name="elu_r")[:pdim, :fdim]
    m = scr.tile([P, CH], F32, tag="elu_m", name="elu_m")[:pdim, :fdim]
    nc.scalar.activation(r, t_ap, AF.Relu)
    nc.vector.tensor_sub(m, t_ap, r)
    nc.scalar.activation(m, m, AF.Exp)
    nc.vector.scalar_tensor_tensor(out_ap, m, -1.0, r, ALU.add, ALU.add)


def _build_program():
    nc = bacc.Bacc(
        "TRN2",
        debug=False,
        enable_asserts=False,
        target_bir_lowering=False,
        num_devices=8,
    )

    xqT = nc.dram_tensor("xqT", [D, S1], F32, kind="ExternalInput").ap()
    xkT = nc.dram_tensor("xkT", [D, S2], F32, kind="ExternalInput").ap()
    xvT = nc.dram_tensor("xvT", [D, S2], F32, kind="ExternalInput").ap()
    zmT = nc.dram_tensor("zmT", [S2, S1], BF16, kind="ExternalInput").ap()
    wq = nc.dram_tensor("wq", [D, D], BF16, kind="ExternalInput").ap()
    wk = nc.dram_tensor("wk", [D, D], BF16, kind="ExternalInput").ap()
    wv = nc.dram_tensor("wv", [D, D], BF16, kind="ExternalInput").ap()
    wo = nc.dram_tensor("wo", [D, D], BF16, kind="ExternalInput").ap()
    bq = nc.dram_tensor("bq", [P, KT], F32, kind="ExternalInput").ap()
    bk = nc.dram_tensor("bk", [P, KT], F32, kind="ExternalInput").ap()
    bv = nc.dram_tensor("bv", [1, D], F32, kind="ExternalInput").ap()
    bo = nc.dram_tensor("bo", [1, D], F32, kind="ExternalInput").ap()
    out = nc.dram_tensor("out", [S1, D], F32, kind="ExternalOutput").ap()

    # DRAM views tiled by 128 partitions
    xqT_t = xqT.rearrange("(t p) s -> p t s", p=P)
    xkT_t = xkT.rearrange("(t p) s -> p t s", p=P)
    xvT_t = xvT.rearrange("(t p) s -> p t s", p=P)
    zmT_t = zmT.rearrange("(t p) s -> p t s", p=P)
    wq_t = wq.rearrange("(t p) f -> p t f", p=P)
    wk_t = wk.rearrange("(t p) f -> p t f", p=P)
    wv_t = wv.rearrange("(t p) f -> p t f", p=P)
    wo_t = wo.rearrange("(t p) f -> p t f", p=P)
    out_t = out.rearrange("(t p) f -> p t f", p=P)

    D2 = 1024  # wide elementwise chunk (2 psum banks)

    def proj(nc, psum_pool, scr, w_s, x_s, b_pp, dst_fn, nft, nch, src_sz):
        """Q^T/K^T-style projection: for each (ft, wide-chunk): accumulate
        8 k matmuls into a [128, D2] psum (two N=512 halves), then elu."""
        for ft in range(nft):
            for c in range(nch):
                ps = psum_pool.tile([P, D2], F32, tag="p1", name="p1")
                for half in range(2):
                    off = c * D2 + half * CH
                    for k in range(KT):
                        nc.tensor.matmul(
                            ps[:, half * CH : (half + 1) * CH],
                            w_s[:, k, ft * P : (ft + 1) * P],
                            x_s[:, k, off : off + CH],
                            start=(k == 0),
                            stop=(k == KT - 1),
                        )
                # elu(ps + b) -> dst
                r = scr.tile([P, D2], F32, tag="elu_r", name="elu_r")
                m = scr.tile([P, D2], F32, tag="elu_m", name="elu_m")
                bias = b_pp(ft) if b_pp is not None else 0.0
                nc.scalar.activation(r[:], ps[:], AF.Relu, bias=bias, scale=1.0)
                if b_pp is not None:
                    nc.vector.scalar_tensor_tensor(
                        m[:], ps[:], b_pp(ft), r[:], ALU.add, ALU.subtract
                    )
                else:
                    nc.vector.tensor_sub(m[:], ps[:], r[:])
                nc.scalar.activation(m[:], m[:], AF.Exp)
                nc.vector.scalar_tensor_tensor(
                    dst_fn(ft, c), m[:], -1.0, r[:], ALU.add, ALU.add
                )

    with tile.TileContext(nc) as tc:
        with (
            tc.tile_pool(name="consts", bufs=1) as consts,
            tc.tile_pool(name="qkv_out", bufs=1) as qkv_out,
        ):
            bq_s = consts.tile([P, KT], F32)
            bk_s = consts.tile([P, KT], F32)
            bv_s = consts.tile([P, D], F32)
            bo_s = consts.tile([P, D], F32)
            nc.sync.dma_start(bq_s[:], bq)
            nc.sync.dma_start(bk_s[:], bk)
            nc.sync.dma_start(bv_s[:], bv.to_broadcast((P, D)))
            nc.sync.dma_start(bo_s[:], bo.to_broadcast((P, D)))

            qT_s = qkv_out.tile([P, KT, S1], BF16)   # head h -> [64*(h%2):, h//2]
            kT_s = qkv_out.tile([P, KT, S2], BF16)
            v_s = qkv_out.tile([P, NT2, H, DEP + 1], BF16)  # V' with ones col
            nc.vector.memset(v_s[:, :, :, DEP : DEP + 1], 1.0)

            # ---------------- Phase 1 ----------------
            with tc.tile_pool(name="p1psum", bufs=3, space="PSUM") as p1psum:
                with (
                    tc.tile_pool(name="wq_pool", bufs=1) as wq_pool,
                    tc.tile_pool(name="xq_pool", bufs=1) as xq_pool,
                    tc.tile_pool(name="stq", bufs=3) as stq,
                    tc.tile_pool(name="scr1q", bufs=3) as scr,
                ):
                    wq_s = wq_pool.tile([P, KT, D], BF16)
                    xq_s = xq_pool.tile([P, KT, S1], BF16)
                    for k in range(KT):
                        nc.sync.dma_start(wq_s[:, k, :], wq_t[:, k, :])
                    for k in range(KT):
                        st = stq.tile([P, S1], F32, tag="xst", name="xst")
                        nc.sync.dma_start(st[:], xqT_t[:, k, :])
                        nc.any.tensor_copy(xq_s[:, k, :], st[:])
                    proj(
                        nc, p1psum, scr, wq_s, xq_s,
                        lambda ft: bq_s[:, ft : ft + 1],
                        lambda ft, c: qT_s[:, ft, c * D2 : (c + 1) * D2],
                        KT, S1 // D2, S1,
                    )
                with (
                    tc.tile_pool(name="wk_pool", bufs=1) as wk_pool,
                    tc.tile_pool(name="xk_pool", bufs=1) as xk_pool,
                    tc.tile_pool(name="stk", bufs=3) as stk,
                    tc.tile_pool(name="scr1k", bufs=3) as scr,
                ):
                    wk_s = wk_pool.tile([P, KT, D], BF16)
                    xk_s = xk_pool.tile([P, KT, S2], BF16)
                    for k in range(KT):
                        nc.sync.dma_start(wk_s[:, k, :], wk_t[:, k, :])
                    for k in range(KT):
                        st = stk.tile([P, S2], F32, tag="xst", name="xst")
                        nc.sync.dma_start(st[:], xkT_t[:, k, :])
                        nc.any.tensor_copy(xk_s[:, k, :], st[:])
                    proj(
                        nc, p1psum, scr, wk_s, xk_s,
                        lambda ft: bk_s[:, ft : ft + 1],
                        lambda ft, c: kT_s[:, ft, c * D2 : (c + 1) * D2],
                        KT, S2 // D2, S2,
                    )
                # V projection (row-major into V')
                with (
                    tc.tile_pool(name="wv_pool", bufs=1) as wv_pool,
                    tc.tile_pool(name="xv_pool", bufs=1) as xv_pool,
                    tc.tile_pool(name="stv", bufs=3) as stv,
                    tc.tile_pool(name="scr1v", bufs=3) as scr,
                ):
                    wv_s = wv_pool.tile([P, KT, D], BF16)
                    xv_s = xv_pool.tile([P, KT, S2], BF16)
                    for k in range(KT):
                        nc.sync.dma_start(wv_s[:, k, :], wv_t[:, k, :])
                    for k in range(KT):
                        st = stv.tile([P, S2], F32, tag="xst", name="xst")
                        nc.sync.dma_start(st[:], xvT_t[:, k, :])
                        nc.any.tensor_copy(xv_s[:, k, :], st[:])
                    for t2 in range(NT2):
                        ps = p1psum.tile([P, D2], F32, tag="p1", name="p1")
                        for half in range(2):
                            for k in range(KT):
                                nc.tensor.matmul(
                                    ps[:, half * CH : (half + 1) * CH],
                                    xv_s[:, k, t2 * P : (t2 + 1) * P],
                                    wv_s[:, k, half * CH : (half + 1) * CH],
                                    start=(k == 0),
                                    stop=(k == KT - 1),
                                )
                        tt = scr.tile([P, D2], F32, tag="vtmp", name="vtmp")
                        nc.vector.tensor_add(tt[:], ps[:], bv_s[:, :])
                        r = scr.tile([P, D2], F32, tag="elu_r", name="elu_r")
                        m = scr.tile([P, D2], F32, tag="elu_m", name="elu_m")
                        nc.scalar.activation(r[:], tt[:], AF.Relu)
                        nc.vector.tensor_sub(m[:], tt[:], r[:])
                        nc.scalar.activation(m[:], m[:], AF.Exp)
                        nc.vector.scalar_tensor_tensor(
                            v_s[:, t2, :, 0:DEP], m[:], -1.0, r[:], ALU.add, ALU.add
                        )

            # ---------------- Phase 2 ----------------
            with tc.tile_pool(name="ctx", bufs=1) as ctxp:
                # ctxT packed: head h -> partitions 64*(h%2).., k-tile h//2
                ctxT_s = ctxp.tile([P, H // 2, S1], BF16)
                with (
                    tc.tile_pool(name="zm", bufs=1) as zm,
                    tc.tile_pool(name="epool", bufs=4) as epool,
                    tc.tile_pool(name="scr2", bufs=3) as scr,
                    tc.tile_pool(name="p2psum", bufs=3, space="PSUM") as p2psum,
                    tc.tile_pool(name="ctxpsum", bufs=1, space="PSUM") as ctxpsum,
                ):
                    zm_s = zm.tile([P, NT2, S1], BF16)
                    for t2 in range(NT2):
                        nc.sync.dma_start(zm_s[:, t2, :], zmT_t[:, t2, :])

                    for h in range(H):
                        base = DEP * (h % 2)
                        pcs = ctxpsum.tile(
                            [DEP + 1, S1], F32, tag="ctxps", name="ctxps"
                        )
                        for t2 in range(NT2):
                            lps = p2psum.tile([P, S1], F32, tag="lps", name="lps")
                            for c in range(S1 // CH):
                                nc.tensor.matmul(
                                    lps[:, c * CH : (c + 1) * CH],
                                    kT_s[base : base + DEP, h // 2,
                                         t2 * P : (t2 + 1) * P],
                                    qT_s[base : base + DEP, h // 2,
                                         c * CH : (c + 1) * CH],
                                    start=True,
                                    stop=True,
                                )
                            e_s = epool.tile([P, S1], BF16, tag="e", name="e_s")
                            nc.scalar.activation(e_s[:], lps[:], AF.Exp, scale=0.125)
                            nc.vector.tensor_mul(e_s[:], e_s[:], zm_s[:, t2, :])
                            for c in range(S1 // CH):
                                nc.tensor.matmul(
                                    pcs[:, c * CH : (c + 1) * CH],
                                    v_s[:, t2, h, :],
                                    e_s[:, c * CH : (c + 1) * CH],
                                    start=(t2 == 0),
                                    stop=(t2 == NT2 - 1),
                                )
                        # drain psum quickly so the next head can accumulate
                        cp = scr.tile([DEP + 1, S1], F32, tag="cp", name="cp")
                        nc.vector.tensor_copy(cp[:], pcs[:])
                        # reciprocal of the denominator row on ACT:
                        # 1/x = exp(-ln(x)); ln+exp+relu share one table set
                        nl = scr.tile([1, S1], F32, tag="nl", name="nl")
                        nc.scalar.activation(nl[:], cp[DEP : DEP + 1, :], AF.Ln)
                        rec = scr.tile([1, S1], F32, tag="rec", name="rec")
                        nc.scalar.activation(rec[:], nl[:], AF.Exp, scale=-1.0)
                        rrep = scr.tile([DEP, S1], F32, tag="rrep", name="rrep")
                        nc.gpsimd.partition_broadcast(rrep[:], rec[:])
                        nc.vector.tensor_mul(
                            ctxT_s[base : base + DEP, h // 2, :],
                            cp[0:DEP, :],
                            rrep[:],
                        )

                # ---------------- Phase 3 ----------------
                with (
                    tc.tile_pool(name="wo_pool", bufs=1) as wo_pool,
                    tc.tile_pool(name="ostage", bufs=3) as ostage,
                    tc.tile_pool(name="scr3", bufs=3) as scr,
                    tc.tile_pool(name="p3psum", bufs=3, space="PSUM") as p3psum,
                ):
                    wo_s = wo_pool.tile([P, KT, D], BF16)
                    for k in range(KT):
                        nc.sync.dma_start(wo_s[:, k, :], wo_t[:, k, :])

                    for t1 in range(NT1):
                        ps = p3psum.tile([P, D2], F32, tag="p3", name="p3")
                        for c in range(D // CH):
                            for kt in range(KT):
                                nc.tensor.matmul(
                                    ps[:, c * CH : (c + 1) * CH],
                                    ctxT_s[:, kt, t1 * P : (t1 + 1) * P],
                                    wo_s[:, kt, c * CH : (c + 1) * CH],
                                    start=(kt == 0),
                                    stop=(kt == KT - 1),
                                )
                        tt = scr.tile([P, D2], F32, tag="otmp", name="otmp")
                        nc.vector.tensor_add(tt[:], ps[:], bo_s[:, :])
                        r = scr.tile([P, D2], F32, tag="elu_r", name="elu_r")
                        m = scr.tile([P, D2], F32, tag="elu_m", name="elu_m")
                        nc.scalar.activation(r[:], tt[:], AF.Relu)
                        nc.vector.tensor_sub(m[:], tt[:], r[:])
                        nc.scalar.activation(m[:], m[:], AF.Exp)
                        ot = ostage.tile([P, D2], F32, tag="ost", name="ost")
                        nc.vector.scalar_tensor_tensor(
                            ot[:], m[:], -1.0, r[:], ALU.add, ALU.add
                        )
                        nc.sync.dma_start(out_t[:, t1, :], ot[:])

    nc.compile()
    return nc


_CACHE = {}


def _get_nc():
    if "nc" not in _CACHE:
        _CACHE["nc"] = _build_program()
    return _CACHE["nc"]


def _prep_inputs(value, key, query, mask, Wq, bq, Wk, bk, Wv, bv, Wo, bo):
    f32 = np.float32
    wq16 = np.ascontiguousarray(Wq, f32).astype(_nbf16)
    wk16 = np.ascontiguousarray(Wk, f32).astype(_nbf16)
    wv16 = np.ascontiguousarray(Wv, f32).astype(_nbf16)
    wo16 = np.ascontiguousarray(Wo, f32).astype(_nbf16)
    bq_r = np.ascontiguousarray(np.asarray(bq, f32).reshape(KT, P).T)
    bk_r = np.ascontiguousarray(np.asarray(bk, f32).reshape(KT, P).T)
    bv_r = np.asarray(bv, f32).reshape(1, D)
    bo_r = np.asarray(bo, f32).reshape(1, D)

    in_maps = []
    for b in range(B):
        xkT = np.ascontiguousarray(np.asarray(key[b], f32).T)
        xvT = np.ascontiguousarray(np.asarray(value[b], f32).T)
        zT = np.ascontiguousarray(
            (1 - np.asarray(mask[b, 0])).T.astype(_nbf16)
        )  # [S2, S1_full]
        qT = np.ascontiguousarray(np.asarray(query[b], f32).T)  # [D, S]
        for hf in range(2):
            sl = slice(hf * S1, (hf + 1) * S1)
            in_maps.append(
                dict(
                    xqT=np.ascontiguousarray(qT[:, sl]),
                    xkT=xkT,
                    xvT=xvT,
                    zmT=np.ascontiguousarray(zT[:, sl]),
                    wq=wq16, wk=wk16, wv=wv16, wo=wo16,
                    bq=bq_r, bk=bk_r, bv=bv_r, bo=bo_r,
                )
            )
    return in_maps


def kernel(value, key, query, mask, Wq, bq, Wk, bk, Wv, bv, Wo, bo, **run_kwargs):
    nc = _get_nc()
    in_maps = _prep_inputs(
        value, key, query, mask, Wq, bq, Wk, bk, Wv, bv, Wo, bo
    )
    res = bass_utils.run_bass_kernel_spmd(
        nc, in_maps, core_ids=list(range(8)), **run_kwargs
    )
    out = np.empty((B, S, D), np.float32)
    for c in range(8):
        b, hf = c // 2, c % 2
        out[b, hf * S1 : (hf + 1) * S1, :] = res.results[c]["out"]
    if run_kwargs:
        _CACHE["last_results"] = res
    return out


# revision 17
# speedup vs baseline: 1.5256x; 1.2156x over previous
"""Trainium2 Bass kernel for CustomMultiHeadAttention.

Problem: B=4, S=2048, D=1024, H=16 heads (depth 64).
  q = elu(query @ Wq + bq); k = elu(key @ Wk + bk); v = elu(value @ Wv + bv)
  logits = (q_h k_h^T)/8 + mask*-1e9 ; attn = softmax ; ctx = attn v
  out = elu(ctx @ Wo + bo)

Sharding: 8 cores = (batch b in 0..3) x (query-half hf in 0..1).
Each core computes out[b, hf*1024:(hf+1)*1024, :] completely
(K/V projections for the batch are duplicated across the half-pair).
No cross-core communication; host only slices/transposes inputs and
concatenates outputs.

Device-side structure (per core), all matmuls bf16 with fp32 PSUM:
  Phase 1: Q^T [D, S1] and K^T [D, S2] feature-major, V row-major
           [S2, D] augmented with a ones column per head (V').
  Phase 2 (per head): L^T tile = K_h^T^T-matmul, i.e. logits transposed
           [s2-part, s1-free]; E = exp(L/8) * zmask^T (multiplicative
           mask; max-subtraction unnecessary: |logits/8| < 3);
           ctx'^T[65, s1] = V'_h^T @ E accumulated over s2 tiles; row 64
           is the softmax denominator (ones column) for free.
  Phase 3: out = elu(ctx^T.T @ Wo + bo) with per-head K=64 accumulation.
"""

import numpy as np
import ml_dtypes

import concourse.bass as bass
import concourse.tile as tile
from concourse import bacc, mybir
from concourse import bass_utils

BF16 = mybir.dt.bfloat16
F32 = mybir.dt.float32
AF = mybir.ActivationFunctionType
ALU = mybir.AluOpType

B, S, D, H = 4, 2048, 1024, 16
DEP = 64          # head depth
S1 = 1024         # query rows per core
S2 = S            # key rows
P = 128
KT = D // P       # 8 contraction tiles of 128
NT1 = S1 // P     # 8 s1 tiles
NT2 = S2 // P     # 16 s2 tiles
CH = 512          # free-dim chunk (one fp32 PSUM bank)

_nbf16 = ml_dtypes.bfloat16


def _elu_from_psum(nc, scr, psum_ap, bias_pp, out_ap):
    """elu(psum + bias) -> out_ap. bias_pp is a per-partition [P,1] f32 AP
    or None. Uses: ACT relu, DVE stt (x - relu(x) = min(x,0)), ACT exp,
    DVE stt ((e - 1) + relu)."""
    pdim, fdim = psum_ap.shape[0], psum_ap.shape[-1]
    r = scr.tile([P, CH], F32, tag="elu_r", name="elu_r")[:pdim, :fdim]
    m = scr.tile([P, CH], F32, tag="elu_m", name="elu_m")[:pdim, :fdim]
    bias = bias_pp if bias_pp is not None else 0.0
    # r = relu(x + bias)
    nc.scalar.activation(r, psum_ap, AF.Relu, bias=bias, scale=1.0)
    # m = (x + bias) - r = min(x + bias, 0)
    if bias_pp is not None:
        nc.vector.scalar_tensor_tensor(m, psum_ap, bias_pp, r, ALU.add, ALU.subtract)
    else:
        nc.vector.tensor_sub(m, psum_ap, r)
    # m = exp(m)  (in place)
    nc.scalar.activation(m, m, AF.Exp)
    # out = (m - 1) + r
    nc.vector.scalar_tensor_tensor(out_ap, m, -1.0, r, ALU.add, ALU.add)


def _elu_from_sbuf(nc, scr, t_ap, out_ap):
    """elu(t) -> out_ap for an SBUF f32 input (bias already added)."""
    pdim, fdim = t_ap.shape[0], t_ap.shape[-1]
    r = scr.tile([P, CH], F32, tag="elu_r", name="elu_r")[:pdim, :fdim]
    m = scr.tile([P, CH], F32, tag="elu_m", name="elu_m")[:pdim, :fdim]
    nc.scalar.activation(r, t_ap, AF.Relu)
    nc.vector.tensor_sub(m, t_ap, r)
    nc.scalar.activation(m, m, AF.Exp)
    nc.vector.scalar_tensor_tensor(out_ap, m, -1.0, r, ALU.add, ALU.add)


def _build_program():
    nc = bacc.Bacc(
        "TRN2",
        debug=False,
        enable_asserts=False,
        target_bir_lowering=False,
        num_devices=8,
    )

    xqT = nc.dram_tensor("xqT", [D, S1], F32, kind="ExternalInput").ap()
    xkT = nc.dram_tensor("xkT", [D, S2], F32, kind="ExternalInput").ap()
    xvT = nc.dram_tensor("xvT", [D, S2], F32, kind="ExternalInput").ap()
    zmT = nc.dram_tensor("zmT", [S2, S1], BF16, kind="ExternalInput").ap()
    wq = nc.dram_tensor("wq", [D, D], BF16, kind="ExternalInput").ap()
    wk = nc.dram_tensor("wk", [D, D], BF16, kind="ExternalInput").ap()
    wv = nc.dram_tensor("wv", [D, D], BF16, kind="ExternalInput").ap()
    wo = nc.dram_tensor("wo", [D, D], BF16, kind="ExternalInput").ap()
    bq = nc.dram_tensor("bq", [P, KT], F32, kind="ExternalInput").ap()
    bk = nc.dram_tensor("bk", [P, KT], F32, kind="ExternalInput").ap()
    bv = nc.dram_tensor("bv", [1, D], F32, kind="ExternalInput").ap()
    bo = nc.dram_tensor("bo", [1, D], F32, kind="ExternalInput").ap()
    out = nc.dram_tensor("out", [S1, D], F32, kind="ExternalOutput").ap()

    # DRAM views tiled by 128 partitions
    xqT_t = xqT.rearrange("(t p) s -> p t s", p=P)
    xkT_t = xkT.rearrange("(t p) s -> p t s", p=P)
    xvT_t = xvT.rearrange("(t p) s -> p t s", p=P)
    zmT_t = zmT.rearrange("(t p) s -> p t s", p=P)
    wq_t = wq.rearrange("(t p) f -> p t f", p=P)
    wk_t = wk.rearrange("(t p) f -> p t f", p=P)
    wv_t = wv.rearrange("(t p) f -> p t f", p=P)
    wo_t = wo.rearrange("(t p) f -> p t f", p=P)
    out_t = out.rearrange("(t p) f -> p t f", p=P)

    D2 = 1024  # wide elementwise chunk (2 psum banks)

    def proj(nc, psum_pool, scr, w_s, x_s, b_pp, dst_fn, nft, nch, src_sz):
        """Q^T/K^T-style projection: for each (ft, wide-chunk): accumulate
        8 k matmuls into a [128, D2] psum (two N=512 halves), then elu."""
        for ft in range(nft):
            for c in range(nch):
                ps = psum_pool.tile([P, D2], F32, tag="p1", name="p1")
                for half in range(2):
                    off = c * D2 + half * CH
                    for k in range(KT):
                        nc.tensor.matmul(
                            ps[:, half * CH : (half + 1) * CH],
                            w_s[:, k, ft * P : (ft + 1) * P],
                            x_s[:, k, off : off + CH],
                            start=(k == 0),
                            stop=(k == KT - 1),
                        )
                # elu(ps + b) -> dst
                r = scr.tile([P, D2], F32, tag="elu_r", name="elu_r")
                m = scr.tile([P, D2], F32, tag="elu_m", name="elu_m")
                bias = b_pp(ft) if b_pp is not None else 0.0
                nc.scalar.activation(r[:], ps[:], AF.Relu, bias=bias, scale=1.0)
                if b_pp is not None:
                    nc.vector.scalar_tensor_tensor(
                        m[:], ps[:], b_pp(ft), r[:], ALU.add, ALU.subtract
                    )
                else:
                    nc.vector.tensor_sub(m[:], ps[:], r[:])
                nc.scalar.activation(m[:], m[:], AF.Exp)
                nc.vector.scalar_tensor_tensor(
                    dst_fn(ft, c), m[:], -1.0, r[:], ALU.add, ALU.add
                )

    with tile.TileContext(nc) as tc:
        with (
            tc.tile_pool(name="consts", bufs=1) as consts,
            tc.tile_pool(name="qkv_out", bufs=1) as qkv_out,
        ):
            bq_s = consts.tile([P, KT], F32)
            bk_s = consts.tile([P, KT], F32)
            bv_s = consts.tile([P, D], F32)
            bo_s = consts.tile([P, D], F32)
            nc.sync.dma_start(bq_s[:], bq)
            nc.sync.dma_start(bk_s[:], bk)
            nc.sync.dma_start(bv_s[:], bv.to_broadcast((P, D)))
            nc.sync.dma_start(bo_s[:], bo.to_broadcast((P, D)))

            qT_s = qkv_out.tile([P, KT, S1], BF16)   # head h -> [64*(h%2):, h//2]
            kT_s = qkv_out.tile([P, KT, S2], BF16)
            v_s = qkv_out.tile([P, NT2, H, DEP + 1], BF16)  # V' with ones col
            nc.vector.memset(v_s[:, :, :, DEP : DEP + 1], 1.0)

            # ---------------- Phase 1 ----------------
            with tc.tile_pool(name="p1psum", bufs=3, space="PSUM") as p1psum:
                with (
                    tc.tile_pool(name="wq_pool", bufs=1) as wq_pool,
                    tc.tile_pool(name="xq_pool", bufs=1) as xq_pool,
                    tc.tile_pool(name="stq", bufs=3) as stq,
                    tc.tile_pool(name="scr1q", bufs=3) as scr,
                ):
                    wq_s = wq_pool.tile([P, KT, D], BF16)
                    xq_s = xq_pool.tile([P, KT, S1], BF16)
                    for k in range(KT):
                        nc.sync.dma_start(wq_s[:, k, :], wq_t[:, k, :])
                    for k in range(KT):
                        st = stq.tile([P, S1], F32, tag="xst", name="xst")
                        nc.sync.dma_start(st[:], xqT_t[:, k, :])
                        nc.any.tensor_copy(xq_s[:, k, :], st[:])
                    proj(
                        nc, p1psum, scr, wq_s, xq_s,
                        lambda ft: bq_s[:, ft : ft + 1],
                        lambda ft, c: qT_s[:, ft, c * D2 : (c + 1) * D2],
                        KT, S1 // D2, S1,
                    )
                with (
                    tc.tile_pool(name="wk_pool", bufs=1) as wk_pool,
                    tc.tile_pool(name="xk_pool", bufs=1) as xk_pool,
                    tc.tile_pool(name="stk", bufs=3) as stk,
                    tc.tile_pool(name="scr1k", bufs=3) as scr,
                ):
                    wk_s = wk_pool.tile([P, KT, D], BF16)
                    xk_s = xk_pool.tile([P, KT, S2], BF16)
                    for k in range(KT):
                        nc.sync.dma_start(wk_s[:, k, :], wk_t[:, k, :])
                    for k in range(KT):
                        st = stk.tile([P, S2], F32, tag="xst", name="xst")
                        nc.sync.dma_start(st[:], xkT_t[:, k, :])
                        nc.any.tensor_copy(xk_s[:, k, :], st[:])
                    proj(
                        nc, p1psum, scr, wk_s, xk_s,
                        lambda ft: bk_s[:, ft : ft + 1],
                        lambda ft, c: kT_s[:, ft, c * D2 : (c + 1) * D2],
                        KT, S2 // D2, S2,
                    )
                # V projection (row-major into V')
                with (
                    tc.tile_pool(name="wv_pool", bufs=1) as wv_pool,
                    tc.tile_pool(name="xv_pool", bufs=1) as xv_pool,
                    tc.tile_pool(name="stv", bufs=3) as stv,
                    tc.tile_pool(name="scr1v", bufs=3) as scr,
                ):
                    wv_s = wv_pool.tile([P, KT, D], BF16)
                    xv_s = xv_pool.tile([P, KT, S2], BF16)
                    for k in range(KT):
                        nc.sync.dma_start(wv_s[:, k, :], wv_t[:, k, :])
                    for k in range(KT):
                        st = stv.tile([P, S2], F32, tag="xst", name="xst")
                        nc.sync.dma_start(st[:], xvT_t[:, k, :])
                        nc.any.tensor_copy(xv_s[:, k, :], st[:])
                    for t2 in range(NT2):
                        ps = p1psum.tile([P, D2], F32, tag="p1", name="p1")
                        for half in range(2):
                            for k in range(KT):
                                nc.tensor.matmul(
                                    ps[:, half * CH : (half + 1) * CH],
                                    xv_s[:, k, t2 * P : (t2 + 1) * P],
                                    wv_s[:, k, half * CH : (half + 1) * CH],
                                    start=(k == 0),
                                    stop=(k == KT - 1),
                                )
                        tt = scr.tile([P, D2], F32, tag="vtmp", name="vtmp")
                        nc.vector.tensor_add(tt[:], ps[:], bv_s[:, :])
                        r = scr.tile([P, D2], F32, tag="elu_r", name="elu_r")
                        m = scr.tile([P, D2], F32, tag="elu_m", name="elu_m")
                        nc.scalar.activation(r[:], tt[:], AF.Relu)
                        nc.vector.tensor_sub(m[:], tt[:], r[:])
                        nc.scalar.activation(m[:], m[:], AF.Exp)
                        nc.vector.scalar_tensor_tensor(
                            v_s[:, t2, :, 0:DEP], m[:], -1.0, r[:], ALU.add, ALU.add
                        )

            # ---------------- Phase 2 ----------------
            with tc.tile_pool(name="ctx", bufs=1) as ctxp:
                # ctxT packed: head h -> partitions 64*(h%2).., k-tile h//2
                ctxT_s = ctxp.tile([P, H // 2, S1], BF16)
                with (
                    tc.tile_pool(name="zm", bufs=1) as zm,
                    tc.tile_pool(name="epool", bufs=4) as epool,
                    tc.tile_pool(name="scr2", bufs=3) as scr,
                    tc.tile_pool(name="p2psum", bufs=3, space="PSUM") as p2psum,
                    tc.tile_pool(name="ctxpsum", bufs=1, space="PSUM") as ctxpsum,
                ):
                    zm_s = zm.tile([P, NT2, S1], BF16)
                    for t2 in range(NT2):
                        nc.sync.dma_start(zm_s[:, t2, :], zmT_t[:, t2, :])

                    dens = zm.tile([H, S1], F32, name="dens")
                    for h in range(H):
                        base = DEP * (h % 2)
                        pcs = ctxpsum.tile(
                            [DEP + 1, S1], F32, tag="ctxps", name="ctxps"
                        )
                        # software-pipelined: issue mm_l(t2) before
                        # mm_ctx(t2-1) so the PE never drains waiting on
                        # the exp->mask chain
                        es = [None] * NT2
                        for t2 in range(NT2 + 1):
                            if t2 < NT2:
                                lps = p2psum.tile([P, S1], F32, tag="lps", name="lps")
                                for c in range(S1 // CH):
                                    nc.tensor.matmul(
                                        lps[:, c * CH : (c + 1) * CH],
                                        kT_s[base : base + DEP, h // 2,
                                             t2 * P : (t2 + 1) * P],
                                        qT_s[base : base + DEP, h // 2,
                                             c * CH : (c + 1) * CH],
                                        start=True,
                                        stop=True,
                                    )
                                e_s = epool.tile([P, S1], BF16, tag="e", name="e_s")
                                nc.scalar.activation(
                                    e_s[:], lps[:], AF.Exp, scale=0.125
                                )
                                nc.vector.tensor_mul(e_s[:], e_s[:], zm_s[:, t2, :])
                                es[t2] = e_s
                            if t2 > 0:
                                for c in range(S1 // CH):
                                    nc.tensor.matmul(
                                        pcs[:, c * CH : (c + 1) * CH],
                                        v_s[:, t2 - 1, h, :],
                                        es[t2 - 1][:, c * CH : (c + 1) * CH],
                                        start=(t2 - 1 == 0),
                                        stop=(t2 - 1 == NT2 - 1),
                                    )
                        # drain psum: unnormalized ctx (bf16) + denominator row
                        nc.vector.tensor_copy(
                            ctxT_s[base : base + DEP, h // 2, :], pcs[0:DEP, :]
                        )
                        dstg = scr.tile([1, S1], F32, tag="dstg", name="dstg")
                        nc.vector.tensor_copy(dstg[:], pcs[DEP : DEP + 1, :])
                        nc.sync.dma_start(dens[h : h + 1, :], dstg[:])

                    # batched softmax denominators: 1/x = exp(-ln(x)) on ACT
                    # (single table-set switch for the whole batch)
                    recs = zm.tile([H, S1], F32, name="recs")
                    nc.scalar.activation(recs[:], dens[:], AF.Ln)
                    nc.scalar.activation(recs[:], recs[:], AF.Exp, scale=-1.0)
                    # bounce through DRAM: DMA supports partition-broadcast
                    # reads from DRAM only
                    recs_d = zm.tile([H, S1], F32, name="recs_d", space="DRAM")
                    nc.sync.dma_start(recs_d[:], recs[:])
                    for h in range(H):
                        base = DEP * (h % 2)
                        # both DVE SBUF inputs must share a base partition:
                        # land the broadcast at the same partitions as ctxT
                        rrep = scr.tile([P, S1], F32, tag="rrep", name="rrep")
                        nc.sync.dma_start(
                            rrep[base : base + DEP, :],
                            recs_d[h : h + 1, :].to_broadcast((DEP, S1)),
                        )
                        nc.vector.tensor_mul(
                            ctxT_s[base : base + DEP, h // 2, :],
                            ctxT_s[base : base + DEP, h // 2, :],
                            rrep[base : base + DEP, :],
                        )

                # ---------------- Phase 3 ----------------
                with (
                    tc.tile_pool(name="wo_pool", bufs=1) as wo_pool,
                    tc.tile_pool(name="ostage", bufs=3) as ostage,
                    tc.tile_pool(name="scr3", bufs=3) as scr,
                    tc.tile_pool(name="p3psum", bufs=3, space="PSUM") as p3psum,
                ):
                    wo_s = wo_pool.tile([P, KT, D], BF16)
                    for k in range(KT):
                        nc.sync.dma_start(wo_s[:, k, :], wo_t[:, k, :])

                    for t1 in range(NT1):
                        ps = p3psum.tile([P, D2], F32, tag="p3", name="p3")
                        for c in range(D // CH):
                            for kt in range(KT):
                                nc.tensor.matmul(
                                    ps[:, c * CH : (c + 1) * CH],
                                    ctxT_s[:, kt, t1 * P : (t1 + 1) * P],
                                    wo_s[:, kt, c * CH : (c + 1) * CH],
                                    start=(kt == 0),
                                    stop=(kt == KT - 1),
                                )
                        tt = scr.tile([P, D2], F32, tag="otmp", name="otmp")
                        nc.vector.tensor_add(tt[:], ps[:], bo_s[:, :])
                        r = scr.tile([P, D2], F32, tag="elu_r", name="elu_r")
                        m = scr.tile([P, D2], F32, tag="elu_m", name="elu_m")
                        nc.scalar.activation(r[:], tt[:], AF.Relu)
                        nc.vector.tensor_sub(m[:], tt[:], r[:])
                        nc.scalar.activation(m[:], m[:], AF.Exp)
                        ot = ostage.tile([P, D2], F32, tag="ost", name="ost")
                        nc.vector.scalar_tensor_tensor(
                            ot[:], m[:], -1.0, r[:], ALU.add, ALU.add
                        )
                        nc.sync.dma_start(out_t[:, t1, :], ot[:])

    nc.compile()
    return nc


_CACHE = {}


def _get_nc():
    if "nc" not in _CACHE:
        _CACHE["nc"] = _build_program()
    return _CACHE["nc"]


def _prep_inputs(value, key, query, mask, Wq, bq, Wk, bk, Wv, bv, Wo, bo):
    f32 = np.float32
    wq16 = np.ascontiguousarray(Wq, f32).astype(_nbf16)
    wk16 = np.ascontiguousarray(Wk, f32).astype(_nbf16)
    wv16 = np.ascontiguousarray(Wv, f32).astype(_nbf16)
    wo16 = np.ascontiguousarray(Wo, f32).astype(_nbf16)
    bq_r = np.ascontiguousarray(np.asarray(bq, f32).reshape(KT, P).T)
    bk_r = np.ascontiguousarray(np.asarray(bk, f32).reshape(KT, P).T)
    bv_r = np.asarray(bv, f32).reshape(1, D)
    bo_r = np.asarray(bo, f32).reshape(1, D)

    in_maps = []
    for b in range(B):
        xkT = np.ascontiguousarray(np.asarray(key[b], f32).T)
        xvT = np.ascontiguousarray(np.asarray(value[b], f32).T)
        zT = np.ascontiguousarray(
            (1 - np.asarray(mask[b, 0])).T.astype(_nbf16)
        )  # [S2, S1_full]
        qT = np.ascontiguousarray(np.asarray(query[b], f32).T)  # [D, S]
        for hf in range(2):
            sl = slice(hf * S1, (hf + 1) * S1)
            in_maps.append(
                dict(
                    xqT=np.ascontiguousarray(qT[:, sl]),
                    xkT=xkT,
                    xvT=xvT,
                    zmT=np.ascontiguousarray(zT[:, sl]),
                    wq=wq16, wk=wk16, wv=wv16, wo=wo16,
                    bq=bq_r, bk=bk_r, bv=bv_r, bo=bo_r,
                )
            )
    return in_maps


def kernel(value, key, query, mask, Wq, bq, Wk, bk, Wv, bv, Wo, bo, **run_kwargs):
    nc = _get_nc()
    in_maps = _prep_inputs(
        value, key, query, mask, Wq, bq, Wk, bk, Wv, bv, Wo, bo
    )
    res = bass_utils.run_bass_kernel_spmd(
        nc, in_maps, core_ids=list(range(8)), **run_kwargs
    )
    out = np.empty((B, S, D), np.float32)
    for c in range(8):
        b, hf = c // 2, c % 2
        out[b, hf * S1 : (hf + 1) * S1, :] = res.results[c]["out"]
    if run_kwargs:
        _CACHE["last_results"] = res
    return out


# revision 19
# speedup vs baseline: 1.6249x; 1.0651x over previous
"""Trainium2 Bass kernel for CustomMultiHeadAttention.

Problem: B=4, S=2048, D=1024, H=16 heads (depth 64).
  q = elu(query @ Wq + bq); k = elu(key @ Wk + bk); v = elu(value @ Wv + bv)
  logits = (q_h k_h^T)/8 + mask*-1e9 ; attn = softmax ; ctx = attn v
  out = elu(ctx @ Wo + bo)

Sharding: 8 cores = (batch b in 0..3) x (query-half hf in 0..1).
Each core computes out[b, hf*1024:(hf+1)*1024, :] completely
(K/V projections for the batch are duplicated across the half-pair).
No cross-core communication; host only slices/transposes inputs and
concatenates outputs.

Device-side structure (per core), all matmuls bf16 with fp32 PSUM:
  Phase 1: Q^T [D, S1] and K^T [D, S2] feature-major, V row-major
           [S2, D] augmented with a ones column per head (V').
  Phase 2 (per head): L^T tile = K_h^T^T-matmul, i.e. logits transposed
           [s2-part, s1-free]; E = exp(L/8) * zmask^T (multiplicative
           mask; max-subtraction unnecessary: |logits/8| < 3);
           ctx'^T[65, s1] = V'_h^T @ E accumulated over s2 tiles; row 64
           is the softmax denominator (ones column) for free.
  Phase 3: out = elu(ctx^T.T @ Wo + bo) with per-head K=64 accumulation.
"""

import numpy as np
import ml_dtypes

import concourse.bass as bass
import concourse.tile as tile
from concourse import bacc, mybir
from concourse import bass_utils

BF16 = mybir.dt.bfloat16
F32 = mybir.dt.float32
AF = mybir.ActivationFunctionType
ALU = mybir.AluOpType

B, S, D, H = 4, 2048, 1024, 16
DEP = 64          # head depth
S1 = 1024         # query rows per core
S2 = S            # key rows
P = 128
KT = D // P       # 8 contraction tiles of 128
NT1 = S1 // P     # 8 s1 tiles
NT2 = S2 // P     # 16 s2 tiles
CH = 512          # free-dim chunk (one fp32 PSUM bank)

_nbf16 = ml_dtypes.bfloat16


def _elu_from_psum(nc, scr, psum_ap, bias_pp, out_ap):
    """elu(psum + bias) -> out_ap. bias_pp is a per-partition [P,1] f32 AP
    or None. Uses: ACT relu, DVE stt (x - relu(x) = min(x,0)), ACT exp,
    DVE stt ((e - 1) + relu)."""
    pdim, fdim = psum_ap.shape[0], psum_ap.shape[-1]
    r = scr.tile([P, CH], F32, tag="elu_r", name="elu_r")[:pdim, :fdim]
    m = scr.tile([P, CH], F32, tag="elu_m", name="elu_m")[:pdim, :fdim]
    bias = bias_pp if bias_pp is not None else 0.0
    # r = relu(x + bias)
    nc.scalar.activation(r, psum_ap, AF.Relu, bias=bias, scale=1.0)
    # m = (x + bias) - r = min(x + bias, 0)
    if bias_pp is not None:
        nc.vector.scalar_tensor_tensor(m, psum_ap, bias_pp, r, ALU.add, ALU.subtract)
    else:
        nc.vector.tensor_sub(m, psum_ap, r)
    # m = exp(m)  (in place)
    nc.scalar.activation(m, m, AF.Exp)
    # out = (m - 1) + r
    nc.vector.scalar_tensor_tensor(out_ap, m, -1.0, r, ALU.add, ALU.add)


def _elu_from_sbuf(nc, scr, t_ap, out_ap):
    """elu(t) -> out_ap for an SBUF f32 input (bias already added)."""
    pdim, fdim = t_ap.shape[0], t_ap.shape[-1]
    r = scr.tile([P, CH], F32, tag="elu_r", name="elu_r")[:pdim, :fdim]
    m = scr.tile([P, CH], F32, tag="elu_m", name="elu_m")[:pdim, :fdim]
    nc.scalar.activation(r, t_ap, AF.Relu)
    nc.vector.tensor_sub(m, t_ap, r)
    nc.scalar.activation(m, m, AF.Exp)
    nc.vector.scalar_tensor_tensor(out_ap, m, -1.0, r, ALU.add, ALU.add)


def _build_program():
    nc = bacc.Bacc(
        "TRN2",
        debug=False,
        enable_asserts=False,
        target_bir_lowering=False,
        num_devices=8,
    )

    xqT = nc.dram_tensor("xqT", [D, S1], BF16, kind="ExternalInput").ap()
    xkT = nc.dram_tensor("xkT", [D, S2], BF16, kind="ExternalInput").ap()
    xvT = nc.dram_tensor("xvT", [D, S2], BF16, kind="ExternalInput").ap()
    zmT = nc.dram_tensor("zmT", [S2, S1], BF16, kind="ExternalInput").ap()
    wq = nc.dram_tensor("wq", [D, D], BF16, kind="ExternalInput").ap()
    wk = nc.dram_tensor("wk", [D, D], BF16, kind="ExternalInput").ap()
    wv = nc.dram_tensor("wv", [D, D], BF16, kind="ExternalInput").ap()
    wo = nc.dram_tensor("wo", [D, D], BF16, kind="ExternalInput").ap()
    bq = nc.dram_tensor("bq", [P, KT], F32, kind="ExternalInput").ap()
    bk = nc.dram_tensor("bk", [P, KT], F32, kind="ExternalInput").ap()
    bv = nc.dram_tensor("bv", [1, D], F32, kind="ExternalInput").ap()
    bo = nc.dram_tensor("bo", [1, D], F32, kind="ExternalInput").ap()
    out = nc.dram_tensor("out", [S1, D], F32, kind="ExternalOutput").ap()

    # DRAM views tiled by 128 partitions
    xqT_t = xqT.rearrange("(t p) s -> p t s", p=P)
    xkT_t = xkT.rearrange("(t p) s -> p t s", p=P)
    xvT_t = xvT.rearrange("(t p) s -> p t s", p=P)
    zmT_t = zmT.rearrange("(t p) s -> p t s", p=P)
    wq_t = wq.rearrange("(t p) f -> p t f", p=P)
    wk_t = wk.rearrange("(t p) f -> p t f", p=P)
    wv_t = wv.rearrange("(t p) f -> p t f", p=P)
    wo_t = wo.rearrange("(t p) f -> p t f", p=P)
    out_t = out.rearrange("(t p) f -> p t f", p=P)

    D2 = 1024  # wide elementwise chunk (2 psum banks)

    def proj(nc, psum_pool, scr, w_s, x_s, b_pp, dst_fn, nft, nch, src_sz):
        """Q^T/K^T-style projection: for each (ft, wide-chunk): accumulate
        8 k matmuls into a [128, D2] psum (two N=512 halves), then elu."""
        for ft in range(nft):
            for c in range(nch):
                ps = psum_pool.tile([P, D2], F32, tag="p1", name="p1")
                for half in range(2):
                    off = c * D2 + half * CH
                    for k in range(KT):
                        nc.tensor.matmul(
                            ps[:, half * CH : (half + 1) * CH],
                            w_s[:, k, ft * P : (ft + 1) * P],
                            x_s[:, k, off : off + CH],
                            start=(k == 0),
                            stop=(k == KT - 1),
                        )
                # elu(ps + b) -> dst
                r = scr.tile([P, D2], F32, tag="elu_r", name="elu_r")
                m = scr.tile([P, D2], F32, tag="elu_m", name="elu_m")
                bias = b_pp(ft) if b_pp is not None else 0.0
                nc.scalar.activation(r[:], ps[:], AF.Relu, bias=bias, scale=1.0)
                if b_pp is not None:
                    nc.vector.scalar_tensor_tensor(
                        m[:], ps[:], b_pp(ft), r[:], ALU.add, ALU.subtract
                    )
                else:
                    nc.vector.tensor_sub(m[:], ps[:], r[:])
                nc.scalar.activation(m[:], m[:], AF.Exp)
                nc.vector.scalar_tensor_tensor(
                    dst_fn(ft, c), m[:], -1.0, r[:], ALU.add, ALU.add
                )

    with tile.TileContext(nc) as tc:
        with (
            tc.tile_pool(name="consts", bufs=1) as consts,
            tc.tile_pool(name="qkv_out", bufs=1) as qkv_out,
        ):
            bq_s = consts.tile([P, KT], F32)
            bk_s = consts.tile([P, KT], F32)
            bv_s = consts.tile([P, D], F32)
            bo_s = consts.tile([P, D], F32)
            nc.sync.dma_start(bq_s[:], bq)
            nc.sync.dma_start(bk_s[:], bk)
            nc.sync.dma_start(bv_s[:], bv.to_broadcast((P, D)))
            nc.sync.dma_start(bo_s[:], bo.to_broadcast((P, D)))

            qT_s = qkv_out.tile([P, KT, S1], BF16)   # head h -> [64*(h%2):, h//2]
            kT_s = qkv_out.tile([P, KT, S2], BF16)
            v_s = qkv_out.tile([P, NT2, H, DEP + 1], BF16)  # V' with ones col
            nc.vector.memset(v_s[:, :, :, DEP : DEP + 1], 1.0)

            # ---------------- Phase 1 ----------------
            with tc.tile_pool(name="p1psum", bufs=3, space="PSUM") as p1psum:
                with (
                    tc.tile_pool(name="wq_pool", bufs=1) as wq_pool,
                    tc.tile_pool(name="xq_pool", bufs=1) as xq_pool,
                    tc.tile_pool(name="scr1q", bufs=3) as scr,
                ):
                    wq_s = wq_pool.tile([P, KT, D], BF16)
                    xq_s = xq_pool.tile([P, KT, S1], BF16)
                    for k in range(KT):
                        nc.sync.dma_start(wq_s[:, k, :], wq_t[:, k, :])
                        nc.sync.dma_start(xq_s[:, k, :], xqT_t[:, k, :])
                    proj(
                        nc, p1psum, scr, wq_s, xq_s,
                        lambda ft: bq_s[:, ft : ft + 1],
                        lambda ft, c: qT_s[:, ft, c * D2 : (c + 1) * D2],
                        KT, S1 // D2, S1,
                    )
                with (
                    tc.tile_pool(name="wk_pool", bufs=1) as wk_pool,
                    tc.tile_pool(name="xk_pool", bufs=1) as xk_pool,
                    tc.tile_pool(name="scr1k", bufs=3) as scr,
                ):
                    wk_s = wk_pool.tile([P, KT, D], BF16)
                    xk_s = xk_pool.tile([P, KT, S2], BF16)
                    for k in range(KT):
                        nc.sync.dma_start(wk_s[:, k, :], wk_t[:, k, :])
                        nc.sync.dma_start(xk_s[:, k, :], xkT_t[:, k, :])
                    proj(
                        nc, p1psum, scr, wk_s, xk_s,
                        lambda ft: bk_s[:, ft : ft + 1],
                        lambda ft, c: kT_s[:, ft, c * D2 : (c + 1) * D2],
                        KT, S2 // D2, S2,
                    )
                # V projection (row-major into V')
                with (
                    tc.tile_pool(name="wv_pool", bufs=1) as wv_pool,
                    tc.tile_pool(name="xv_pool", bufs=1) as xv_pool,
                    tc.tile_pool(name="scr1v", bufs=3) as scr,
                ):
                    wv_s = wv_pool.tile([P, KT, D], BF16)
                    xv_s = xv_pool.tile([P, KT, S2], BF16)
                    for k in range(KT):
                        nc.sync.dma_start(wv_s[:, k, :], wv_t[:, k, :])
                        nc.sync.dma_start(xv_s[:, k, :], xvT_t[:, k, :])
                    for t2 in range(NT2):
                        ps = p1psum.tile([P, D2], F32, tag="p1", name="p1")
                        for half in range(2):
                            for k in range(KT):
                                nc.tensor.matmul(
                                    ps[:, half * CH : (half + 1) * CH],
                                    xv_s[:, k, t2 * P : (t2 + 1) * P],
                                    wv_s[:, k, half * CH : (half + 1) * CH],
                                    start=(k == 0),
                                    stop=(k == KT - 1),
                                )
                        tt = scr.tile([P, D2], F32, tag="vtmp", name="vtmp")
                        nc.vector.tensor_add(tt[:], ps[:], bv_s[:, :])
                        r = scr.tile([P, D2], F32, tag="elu_r", name="elu_r")
                        m = scr.tile([P, D2], F32, tag="elu_m", name="elu_m")
                        nc.scalar.activation(r[:], tt[:], AF.Relu)
                        nc.vector.tensor_sub(m[:], tt[:], r[:])
                        nc.scalar.activation(m[:], m[:], AF.Exp)
                        nc.vector.scalar_tensor_tensor(
                            v_s[:, t2, :, 0:DEP], m[:], -1.0, r[:], ALU.add, ALU.add
                        )

            # ---------------- Phase 2 ----------------
            with tc.tile_pool(name="ctx", bufs=1) as ctxp:
                # ctxT packed: head h -> partitions 64*(h%2).., k-tile h//2
                ctxT_s = ctxp.tile([P, H // 2, S1], BF16)
                with (
                    tc.tile_pool(name="zm", bufs=1) as zm,
                    tc.tile_pool(name="epool", bufs=4) as epool,
                    tc.tile_pool(name="scr2", bufs=3) as scr,
                    tc.tile_pool(name="p2psum", bufs=2, space="PSUM") as p2psum,
                    tc.tile_pool(name="ctxpsum", bufs=4, space="PSUM") as ctxpsum,
                ):
                    zm_s = zm.tile([P, NT2, S1], BF16)
                    for t2 in range(NT2):
                        nc.sync.dma_start(zm_s[:, t2, :], zmT_t[:, t2, :])

                    dens = zm.tile([H, S1], F32, name="dens")
                    NCH2 = S1 // CH
                    for c in range(NCH2):
                        csl = slice(c * CH, (c + 1) * CH)
                        for hp in range(H // 2):
                            heads = (2 * hp, 2 * hp + 1)
                            pcs = [
                                ctxpsum.tile([DEP + 1, CH], F32, tag="ctxps",
                                             name="ctxps")
                                for _ in heads
                            ]
                            es = [None] * NT2
                            for t2 in range(NT2 + 1):
                                if t2 < NT2:
                                    lps = p2psum.tile([P, 2, CH], F32, tag="lps",
                                                      name="lps")
                                    for i, h in enumerate(heads):
                                        base = DEP * (h % 2)
                                        nc.tensor.matmul(
                                            lps[:, i, :],
                                            kT_s[base : base + DEP, h // 2,
                                                 t2 * P : (t2 + 1) * P],
                                            qT_s[base : base + DEP, h // 2, csl],
                                            start=True,
                                            stop=True,
                                        )
                                    e_s = epool.tile([P, 2, CH], BF16, tag="e",
                                                     name="e_s")
                                    nc.scalar.activation(
                                        e_s[:], lps[:], AF.Exp, scale=0.125
                                    )
                                    zc = zm_s[:, t2, csl]
                                    zb = bass.AP(
                                        tensor=zc.tensor, offset=zc.offset,
                                        ap=[zc.ap[0], [0, 2], zc.ap[1]],
                                    )
                                    nc.vector.tensor_mul(e_s[:], e_s[:], zb)
                                    es[t2] = e_s
                                if t2 > 0:
                                    for i, h in enumerate(heads):
                                        nc.tensor.matmul(
                                            pcs[i][:],
                                            v_s[:, t2 - 1, h, :],
                                            es[t2 - 1][:, i, :],
                                            start=(t2 - 1 == 0),
                                            stop=(t2 - 1 == NT2 - 1),
                                        )
                            for i, h in enumerate(heads):
                                base = DEP * (h % 2)
                                nc.vector.tensor_copy(
                                    ctxT_s[base : base + DEP, h // 2, csl],
                                    pcs[i][0:DEP, :],
                                )
                                dstg = scr.tile([1, CH], F32, tag="dstg",
                                                name="dstg")
                                nc.vector.tensor_copy(
                                    dstg[:], pcs[i][DEP : DEP + 1, :]
                                )
                                nc.sync.dma_start(dens[h : h + 1, csl], dstg[:])

                    # batched softmax denominators: 1/x = exp(-ln(x)) on ACT
                    # (single table-set switch for the whole batch)
                    recs = zm.tile([H, S1], F32, name="recs")
                    nc.scalar.activation(recs[:], dens[:], AF.Ln)
                    nc.scalar.activation(recs[:], recs[:], AF.Exp, scale=-1.0)
                    # bounce through DRAM: DMA supports partition-broadcast
                    # reads from DRAM only
                    recs_d = zm.tile([H, S1], F32, name="recs_d", space="DRAM")
                    nc.sync.dma_start(recs_d[:], recs[:])
                    for h in range(H):
                        base = DEP * (h % 2)
                        # both DVE SBUF inputs must share a base partition:
                        # land the broadcast at the same partitions as ctxT
                        rrep = scr.tile([P, S1], F32, tag="rrep", name="rrep")
                        nc.sync.dma_start(
                            rrep[base : base + DEP, :],
                            recs_d[h : h + 1, :].to_broadcast((DEP, S1)),
                        )
                        nc.vector.tensor_mul(
                            ctxT_s[base : base + DEP, h // 2, :],
                            ctxT_s[base : base + DEP, h // 2, :],
                            rrep[base : base + DEP, :],
                        )

                # ---------------- Phase 3 ----------------
                with (
                    tc.tile_pool(name="wo_pool", bufs=1) as wo_pool,
                    tc.tile_pool(name="ostage", bufs=3) as ostage,
                    tc.tile_pool(name="scr3", bufs=3) as scr,
                    tc.tile_pool(name="p3psum", bufs=3, space="PSUM") as p3psum,
                ):
                    wo_s = wo_pool.tile([P, KT, D], BF16)
                    for k in range(KT):
                        nc.sync.dma_start(wo_s[:, k, :], wo_t[:, k, :])

                    for t1 in range(NT1):
                        ps = p3psum.tile([P, D2], F32, tag="p3", name="p3")
                        for c in range(D // CH):
                            for kt in range(KT):
                                nc.tensor.matmul(
                                    ps[:, c * CH : (c + 1) * CH],
                                    ctxT_s[:, kt, t1 * P : (t1 + 1) * P],
                                    wo_s[:, kt, c * CH : (c + 1) * CH],
                                    start=(kt == 0),
                                    stop=(kt == KT - 1),
                                )
                        tt = scr.tile([P, D2], F32, tag="otmp", name="otmp")
                        nc.vector.tensor_add(tt[:], ps[:], bo_s[:, :])
                        r = scr.tile([P, D2], F32, tag="elu_r", name="elu_r")
                        m = scr.tile([P, D2], F32, tag="elu_m", name="elu_m")
                        nc.scalar.activation(r[:], tt[:], AF.Relu)
                        nc.vector.tensor_sub(m[:], tt[:], r[:])
                        nc.scalar.activation(m[:], m[:], AF.Exp)
                        ot = ostage.tile([P, D2], F32, tag="ost", name="ost")
                        nc.vector.scalar_tensor_tensor(
                            ot[:], m[:], -1.0, r[:], ALU.add, ALU.add
                        )
                        nc.sync.dma_start(out_t[:, t1, :], ot[:])

    nc.compile()
    return nc


_CACHE = {}


def _get_nc():
    if "nc" not in _CACHE:
        _CACHE["nc"] = _build_program()
    return _CACHE["nc"]


def _prep_inputs(value, key, query, mask, Wq, bq, Wk, bk, Wv, bv, Wo, bo):
    f32 = np.float32
    wq16 = np.ascontiguousarray(Wq, f32).astype(_nbf16)
    wk16 = np.ascontiguousarray(Wk, f32).astype(_nbf16)
    wv16 = np.ascontiguousarray(Wv, f32).astype(_nbf16)
    wo16 = np.ascontiguousarray(Wo, f32).astype(_nbf16)
    bq_r = np.ascontiguousarray(np.asarray(bq, f32).reshape(KT, P).T)
    bk_r = np.ascontiguousarray(np.asarray(bk, f32).reshape(KT, P).T)
    bv_r = np.asarray(bv, f32).reshape(1, D)
    bo_r = np.asarray(bo, f32).reshape(1, D)

    in_maps = []
    for b in range(B):
        xkT = np.ascontiguousarray(np.asarray(key[b], f32).T.astype(_nbf16))
        xvT = np.ascontiguousarray(np.asarray(value[b], f32).T.astype(_nbf16))
        zT = np.ascontiguousarray(
            (1 - np.asarray(mask[b, 0])).T.astype(_nbf16)
        )  # [S2, S1_full]
        qT = np.ascontiguousarray(np.asarray(query[b], f32).T.astype(_nbf16))  # [D, S]
        for hf in range(2):
            sl = slice(hf * S1, (hf + 1) * S1)
            in_maps.append(
                dict(
                    xqT=np.ascontiguousarray(qT[:, sl]),
                    xkT=xkT,
                    xvT=xvT,
                    zmT=np.ascontiguousarray(zT[:, sl]),
                    wq=wq16, wk=wk16, wv=wv16, wo=wo16,
                    bq=bq_r, bk=bk_r, bv=bv_r, bo=bo_r,
                )
            )
    return in_maps


def kernel(value, key, query, mask, Wq, bq, Wk, bk, Wv, bv, Wo, bo, **run_kwargs):
    nc = _get_nc()
    in_maps = _prep_inputs(
        value, key, query, mask, Wq, bq, Wk, bk, Wv, bv, Wo, bo
    )
    res = bass_utils.run_bass_kernel_spmd(
        nc, in_maps, core_ids=list(range(8)), **run_kwargs
    )
    out = np.empty((B, S, D), np.float32)
    for c in range(8):
        b, hf = c // 2, c % 2
        out[b, hf * S1 : (hf + 1) * S1, :] = res.results[c]["out"]
    if run_kwargs:
        _CACHE["last_results"] = res
    return out


# revision 20
# speedup vs baseline: 1.6505x; 1.0157x over previous
"""Trainium2 Bass kernel for CustomMultiHeadAttention.

Problem: B=4, S=2048, D=1024, H=16 heads (depth 64).
  q = elu(query @ Wq + bq); k = elu(key @ Wk + bk); v = elu(value @ Wv + bv)
  logits = (q_h k_h^T)/8 + mask*-1e9 ; attn = softmax ; ctx = attn v
  out = elu(ctx @ Wo + bo)

Sharding: 8 cores = (batch b in 0..3) x (query-half hf in 0..1).
Each core computes out[b, hf*1024:(hf+1)*1024, :] completely
(K/V projections for the batch are duplicated across the half-pair).
No cross-core communication; host only slices/transposes inputs and
concatenates outputs.

Device-side structure (per core), all matmuls bf16 with fp32 PSUM:
  Phase 1: Q^T [D, S1] and K^T [D, S2] feature-major, V row-major
           [S2, D] augmented with a ones column per head (V').
  Phase 2 (per head): L^T tile = K_h^T^T-matmul, i.e. logits transposed
           [s2-part, s1-free]; E = exp(L/8) * zmask^T (multiplicative
           mask; max-subtraction unnecessary: |logits/8| < 3);
           ctx'^T[65, s1] = V'_h^T @ E accumulated over s2 tiles; row 64
           is the softmax denominator (ones column) for free.
  Phase 3: out = elu(ctx^T.T @ Wo + bo) with per-head K=64 accumulation.
"""

import numpy as np
import ml_dtypes

import concourse.bass as bass
import concourse.tile as tile
from concourse import bacc, mybir
from concourse import bass_utils

BF16 = mybir.dt.bfloat16
F32 = mybir.dt.float32
AF = mybir.ActivationFunctionType
ALU = mybir.AluOpType

B, S, D, H = 4, 2048, 1024, 16
DEP = 64          # head depth
S1 = 1024         # query rows per core
S2 = S            # key rows
P = 128
KT = D // P       # 8 contraction tiles of 128
NT1 = S1 // P     # 8 s1 tiles
NT2 = S2 // P     # 16 s2 tiles
CH = 512          # free-dim chunk (one fp32 PSUM bank)

_nbf16 = ml_dtypes.bfloat16


def _elu_from_psum(nc, scr, psum_ap, bias_pp, out_ap):
    """elu(psum + bias) -> out_ap. bias_pp is a per-partition [P,1] f32 AP
    or None. Uses: ACT relu, DVE stt (x - relu(x) = min(x,0)), ACT exp,
    DVE stt ((e - 1) + relu)."""
    pdim, fdim = psum_ap.shape[0], psum_ap.shape[-1]
    r = scr.tile([P, CH], F32, tag="elu_r", name="elu_r")[:pdim, :fdim]
    m = scr.tile([P, CH], F32, tag="elu_m", name="elu_m")[:pdim, :fdim]
    bias = bias_pp if bias_pp is not None else 0.0
    # r = relu(x + bias)
    nc.scalar.activation(r, psum_ap, AF.Relu, bias=bias, scale=1.0)
    # m = (x + bias) - r = min(x + bias, 0)
    if bias_pp is not None:
        nc.vector.scalar_tensor_tensor(m, psum_ap, bias_pp, r, ALU.add, ALU.subtract)
    else:
        nc.vector.tensor_sub(m, psum_ap, r)
    # m = exp(m)  (in place)
    nc.scalar.activation(m, m, AF.Exp)
    # out = (m - 1) + r
    nc.vector.scalar_tensor_tensor(out_ap, m, -1.0, r, ALU.add, ALU.add)


def _elu_from_sbuf(nc, scr, t_ap, out_ap):
    """elu(t) -> out_ap for an SBUF f32 input (bias already added)."""
    pdim, fdim = t_ap.shape[0], t_ap.shape[-1]
    r = scr.tile([P, CH], F32, tag="elu_r", name="elu_r")[:pdim, :fdim]
    m = scr.tile([P, CH], F32, tag="elu_m", name="elu_m")[:pdim, :fdim]
    nc.scalar.activation(r, t_ap, AF.Relu)
    nc.vector.tensor_sub(m, t_ap, r)
    nc.scalar.activation(m, m, AF.Exp)
    nc.vector.scalar_tensor_tensor(out_ap, m, -1.0, r, ALU.add, ALU.add)


def _build_program():
    nc = bacc.Bacc(
        "TRN2",
        debug=False,
        enable_asserts=False,
        target_bir_lowering=False,
        num_devices=8,
    )

    xqT = nc.dram_tensor("xqT", [D, S1], BF16, kind="ExternalInput").ap()
    xkT = nc.dram_tensor("xkT", [D, S2], BF16, kind="ExternalInput").ap()
    xvT = nc.dram_tensor("xvT", [D, S2], BF16, kind="ExternalInput").ap()
    zmT = nc.dram_tensor("zmT", [S2, S1], BF16, kind="ExternalInput").ap()
    wq = nc.dram_tensor("wq", [D, D], BF16, kind="ExternalInput").ap()
    wk = nc.dram_tensor("wk", [D, D], BF16, kind="ExternalInput").ap()
    wv = nc.dram_tensor("wv", [D, D], BF16, kind="ExternalInput").ap()
    wo = nc.dram_tensor("wo", [D, D], BF16, kind="ExternalInput").ap()
    bq = nc.dram_tensor("bq", [P, KT], F32, kind="ExternalInput").ap()
    bk = nc.dram_tensor("bk", [P, KT], F32, kind="ExternalInput").ap()
    bv = nc.dram_tensor("bv", [1, D], F32, kind="ExternalInput").ap()
    bo = nc.dram_tensor("bo", [1, D], F32, kind="ExternalInput").ap()
    out = nc.dram_tensor("out", [S1, D], F32, kind="ExternalOutput").ap()

    # DRAM views tiled by 128 partitions
    xqT_t = xqT.rearrange("(t p) s -> p t s", p=P)
    xkT_t = xkT.rearrange("(t p) s -> p t s", p=P)
    xvT_t = xvT.rearrange("(t p) s -> p t s", p=P)
    zmT_t = zmT.rearrange("(t p) s -> p t s", p=P)
    wq_t = wq.rearrange("(t p) f -> p t f", p=P)
    wk_t = wk.rearrange("(t p) f -> p t f", p=P)
    wv_t = wv.rearrange("(t p) f -> p t f", p=P)
    wo_t = wo.rearrange("(t p) f -> p t f", p=P)
    out_t = out.rearrange("(t p) f -> p t f", p=P)

    D2 = 1024  # wide elementwise chunk (2 psum banks)

    def proj(nc, psum_pool, scr, w_s, x_s, b_pp, dst_fn, nft, nch, src_sz):
        """Q^T/K^T-style projection: for each (ft, wide-chunk): accumulate
        8 k matmuls into a [128, D2] psum (two N=512 halves), then elu."""
        for ft in range(nft):
            for c in range(nch):
                ps = psum_pool.tile([P, D2], F32, tag="p1", name="p1")
                for half in range(2):
                    off = c * D2 + half * CH
                    for k in range(KT):
                        nc.tensor.matmul(
                            ps[:, half * CH : (half + 1) * CH],
                            w_s[:, k, ft * P : (ft + 1) * P],
                            x_s[:, k, off : off + CH],
                            start=(k == 0),
                            stop=(k == KT - 1),
                        )
                # elu(ps + b) -> dst
                r = scr.tile([P, D2], F32, tag="elu_r", name="elu_r")
                m = scr.tile([P, D2], F32, tag="elu_m", name="elu_m")
                bias = b_pp(ft) if b_pp is not None else 0.0
                nc.scalar.activation(r[:], ps[:], AF.Relu, bias=bias, scale=1.0)
                if b_pp is not None:
                    nc.vector.scalar_tensor_tensor(
                        m[:], ps[:], b_pp(ft), r[:], ALU.add, ALU.subtract
                    )
                else:
                    nc.vector.tensor_sub(m[:], ps[:], r[:])
                nc.scalar.activation(m[:], m[:], AF.Exp)
                nc.vector.scalar_tensor_tensor(
                    dst_fn(ft, c), m[:], -1.0, r[:], ALU.add, ALU.add
                )

    with tile.TileContext(nc) as tc:
        with (
            tc.tile_pool(name="consts", bufs=1) as consts,
            tc.tile_pool(name="qkv_out", bufs=1) as qkv_out,
        ):
            bq_s = consts.tile([P, KT], F32)
            bk_s = consts.tile([P, KT], F32)
            bv_s = consts.tile([P, D], F32)
            bo_s = consts.tile([P, D], F32)
            nc.sync.dma_start(bq_s[:], bq)
            nc.sync.dma_start(bk_s[:], bk)
            nc.sync.dma_start(bv_s[:], bv.to_broadcast((P, D)))
            nc.sync.dma_start(bo_s[:], bo.to_broadcast((P, D)))

            qT_s = qkv_out.tile([P, KT, S1], BF16)   # head h -> [64*(h%2):, h//2]
            kT_s = qkv_out.tile([P, KT, S2], BF16)
            v_s = qkv_out.tile([P, NT2, H, DEP + 1], BF16)  # V' with ones col
            nc.vector.memset(v_s[:, :, :, DEP : DEP + 1], 1.0)

            # ---------------- Phase 1 ----------------
            with tc.tile_pool(name="p1psum", bufs=3, space="PSUM") as p1psum:
                with (
                    tc.tile_pool(name="wq_pool", bufs=1) as wq_pool,
                    tc.tile_pool(name="xq_pool", bufs=1) as xq_pool,
                    tc.tile_pool(name="scr1q", bufs=3) as scr,
                ):
                    wq_s = wq_pool.tile([P, KT, D], BF16)
                    xq_s = xq_pool.tile([P, KT, S1], BF16)
                    for k in range(KT):
                        nc.sync.dma_start(wq_s[:, k, :], wq_t[:, k, :])
                        nc.sync.dma_start(xq_s[:, k, :], xqT_t[:, k, :])
                    proj(
                        nc, p1psum, scr, wq_s, xq_s,
                        lambda ft: bq_s[:, ft : ft + 1],
                        lambda ft, c: qT_s[:, ft, c * D2 : (c + 1) * D2],
                        KT, S1 // D2, S1,
                    )
                with (
                    tc.tile_pool(name="wk_pool", bufs=1) as wk_pool,
                    tc.tile_pool(name="xk_pool", bufs=1) as xk_pool,
                    tc.tile_pool(name="scr1k", bufs=3) as scr,
                ):
                    wk_s = wk_pool.tile([P, KT, D], BF16)
                    xk_s = xk_pool.tile([P, KT, S2], BF16)
                    for k in range(KT):
                        nc.sync.dma_start(wk_s[:, k, :], wk_t[:, k, :])
                        nc.sync.dma_start(xk_s[:, k, :], xkT_t[:, k, :])
                    proj(
                        nc, p1psum, scr, wk_s, xk_s,
                        lambda ft: bk_s[:, ft : ft + 1],
                        lambda ft, c: kT_s[:, ft, c * D2 : (c + 1) * D2],
                        KT, S2 // D2, S2,
                    )
                # V projection (row-major into V')
                with (
                    tc.tile_pool(name="wv_pool", bufs=1) as wv_pool,
                    tc.tile_pool(name="xv_pool", bufs=1) as xv_pool,
                    tc.tile_pool(name="scr1v", bufs=3) as scr,
                ):
                    wv_s = wv_pool.tile([P, KT, D], BF16)
                    xv_s = xv_pool.tile([P, KT, S2], BF16)
                    for k in range(KT):
                        nc.sync.dma_start(wv_s[:, k, :], wv_t[:, k, :])
                        nc.sync.dma_start(xv_s[:, k, :], xvT_t[:, k, :])
                    for t2 in range(NT2):
                        ps = p1psum.tile([P, D2], F32, tag="p1", name="p1")
                        for half in range(2):
                            for k in range(KT):
                                nc.tensor.matmul(
                                    ps[:, half * CH : (half + 1) * CH],
                                    xv_s[:, k, t2 * P : (t2 + 1) * P],
                                    wv_s[:, k, half * CH : (half + 1) * CH],
                                    start=(k == 0),
                                    stop=(k == KT - 1),
                                )
                        tt = scr.tile([P, D2], F32, tag="vtmp", name="vtmp")
                        nc.vector.tensor_add(tt[:], ps[:], bv_s[:, :])
                        r = scr.tile([P, D2], F32, tag="elu_r", name="elu_r")
                        m = scr.tile([P, D2], F32, tag="elu_m", name="elu_m")
                        nc.scalar.activation(r[:], tt[:], AF.Relu)
                        nc.vector.tensor_sub(m[:], tt[:], r[:])
                        nc.scalar.activation(m[:], m[:], AF.Exp)
                        nc.vector.scalar_tensor_tensor(
                            v_s[:, t2, :, 0:DEP], m[:], -1.0, r[:], ALU.add, ALU.add
                        )

            # ---------------- Phase 2 ----------------
            with tc.tile_pool(name="ctx", bufs=1) as ctxp:
                # ctxT packed: head h -> partitions 64*(h%2).., k-tile h//2
                ctxT_s = ctxp.tile([P, H // 2, S1], BF16)
                with (
                    tc.tile_pool(name="zm", bufs=1) as zm,
                    tc.tile_pool(name="epool", bufs=4) as epool,
                    tc.tile_pool(name="scr2", bufs=3) as scr,
                    tc.tile_pool(name="p2psum", bufs=2, space="PSUM") as p2psum,
                    tc.tile_pool(name="ctxpsum", bufs=4, space="PSUM") as ctxpsum,
                ):
                    zm_s = zm.tile([P, NT2, S1], BF16)
                    for t2 in range(NT2):
                        nc.sync.dma_start(zm_s[:, t2, :], zmT_t[:, t2, :])

                    dens = zm.tile([H, S1], F32, name="dens")
                    recs = zm.tile([H, S1], F32, name="recs")
                    recs_d = zm.tile([H, S1], F32, name="recs_d", space="DRAM")
                    NCH2 = S1 // CH
                    for c in range(NCH2):
                        csl = slice(c * CH, (c + 1) * CH)
                        steps = [(hp, t2) for hp in range(H // 2)
                                 for t2 in range(NT2)]
                        pcs_map = {}
                        es_map = {}
                        for idx in range(len(steps) + 1):
                            if idx < len(steps):
                                hp, t2 = steps[idx]
                                heads = (2 * hp, 2 * hp + 1)
                                if t2 == 0:
                                    pcs_map[hp] = [
                                        ctxpsum.tile([DEP + 1, CH], F32,
                                                     tag="ctxps", name="ctxps")
                                        for _ in heads
                                    ]
                                lps = p2psum.tile([P, 2, CH], F32, tag="lps",
                                                  name="lps")
                                for i, h in enumerate(heads):
                                    base = DEP * (h % 2)
                                    nc.tensor.matmul(
                                        lps[:, i, :],
                                        kT_s[base : base + DEP, h // 2,
                                             t2 * P : (t2 + 1) * P],
                                        qT_s[base : base + DEP, h // 2, csl],
                                        start=True,
                                        stop=True,
                                    )
                                e_s = epool.tile([P, 2, CH], BF16, tag="e",
                                                 name="e_s")
                                nc.scalar.activation(
                                    e_s[:], lps[:], AF.Exp, scale=0.125
                                )
                                zc = zm_s[:, t2, csl]
                                zb = bass.AP(
                                    tensor=zc.tensor, offset=zc.offset,
                                    ap=[zc.ap[0], [0, 2], zc.ap[1]],
                                )
                                nc.vector.tensor_mul(e_s[:], e_s[:], zb)
                                es_map[(hp, t2)] = e_s
                            if idx > 0:
                                hp0, t20 = steps[idx - 1]
                                heads0 = (2 * hp0, 2 * hp0 + 1)
                                e_prev = es_map.pop((hp0, t20))
                                for i, h in enumerate(heads0):
                                    nc.tensor.matmul(
                                        pcs_map[hp0][i][:],
                                        v_s[:, t20, h, :],
                                        e_prev[:, i, :],
                                        start=(t20 == 0),
                                        stop=(t20 == NT2 - 1),
                                    )
                                if t20 == NT2 - 1:
                                    for i, h in enumerate(heads0):
                                        base = DEP * (h % 2)
                                        nc.vector.tensor_copy(
                                            ctxT_s[base : base + DEP, h // 2, csl],
                                            pcs_map[hp0][i][0:DEP, :],
                                        )
                                        dstg = scr.tile([1, CH], F32, tag="dstg",
                                                        name="dstg")
                                        nc.vector.tensor_copy(
                                            dstg[:], pcs_map[hp0][i][DEP : DEP + 1, :]
                                        )
                                        nc.sync.dma_start(
                                            dens[h : h + 1, csl], dstg[:]
                                        )
                                    del pcs_map[hp0]
                        # normalize this chunk (overlaps the next chunk's work):
                        # 1/x = exp(-ln(x)) on ACT, broadcast via DRAM bounce
                        nc.scalar.activation(recs[:, csl], dens[:, csl], AF.Ln)
                        nc.scalar.activation(
                            recs[:, csl], recs[:, csl], AF.Exp, scale=-1.0
                        )
                        nc.sync.dma_start(recs_d[:, csl], recs[:, csl])
                        for h in range(H):
                            base = DEP * (h % 2)
                            rrep = scr.tile([P, CH], F32, tag="rrep", name="rrep")
                            nc.sync.dma_start(
                                rrep[base : base + DEP, :],
                                recs_d[h : h + 1, csl].to_broadcast((DEP, CH)),
                            )
                            nc.vector.tensor_mul(
                                ctxT_s[base : base + DEP, h // 2, csl],
                                ctxT_s[base : base + DEP, h // 2, csl],
                                rrep[base : base + DEP, :],
                            )

                # ---------------- Phase 3 ----------------
                with (
                    tc.tile_pool(name="wo_pool", bufs=1) as wo_pool,
                    tc.tile_pool(name="ostage", bufs=3) as ostage,
                    tc.tile_pool(name="scr3", bufs=3) as scr,
                    tc.tile_pool(name="p3psum", bufs=3, space="PSUM") as p3psum,
                ):
                    wo_s = wo_pool.tile([P, KT, D], BF16)
                    for k in range(KT):
                        nc.sync.dma_start(wo_s[:, k, :], wo_t[:, k, :])

                    for t1 in range(NT1):
                        ps = p3psum.tile([P, D2], F32, tag="p3", name="p3")
                        for c in range(D // CH):
                            for kt in range(KT):
                                nc.tensor.matmul(
                                    ps[:, c * CH : (c + 1) * CH],
                                    ctxT_s[:, kt, t1 * P : (t1 + 1) * P],
                                    wo_s[:, kt, c * CH : (c + 1) * CH],
                                    start=(kt == 0),
                                    stop=(kt == KT - 1),
                                )
                        tt = scr.tile([P, D2], F32, tag="otmp", name="otmp")
                        nc.vector.tensor_add(tt[:], ps[:], bo_s[:, :])
                        r = scr.tile([P, D2], F32, tag="elu_r", name="elu_r")
                        m = scr.tile([P, D2], F32, tag="elu_m", name="elu_m")
                        nc.scalar.activation(r[:], tt[:], AF.Relu)
                        nc.vector.tensor_sub(m[:], tt[:], r[:])
                        nc.scalar.activation(m[:], m[:], AF.Exp)
                        ot = ostage.tile([P, D2], F32, tag="ost", name="ost")
                        nc.vector.scalar_tensor_tensor(
                            ot[:], m[:], -1.0, r[:], ALU.add, ALU.add
                        )
                        nc.sync.dma_start(out_t[:, t1, :], ot[:])

    nc.compile()
    return nc


_CACHE = {}


def _get_nc():
    if "nc" not in _CACHE:
        _CACHE["nc"] = _build_program()
    return _CACHE["nc"]


def _prep_inputs(value, key, query, mask, Wq, bq, Wk, bk, Wv, bv, Wo, bo):
    f32 = np.float32
    wq16 = np.ascontiguousarray(Wq, f32).astype(_nbf16)
    wk16 = np.ascontiguousarray(Wk, f32).astype(_nbf16)
    wv16 = np.ascontiguousarray(Wv, f32).astype(_nbf16)
    wo16 = np.ascontiguousarray(Wo, f32).astype(_nbf16)
    bq_r = np.ascontiguousarray(np.asarray(bq, f32).reshape(KT, P).T)
    bk_r = np.ascontiguousarray(np.asarray(bk, f32).reshape(KT, P).T)
    bv_r = np.asarray(bv, f32).reshape(1, D)
    bo_r = np.asarray(bo, f32).reshape(1, D)

    in_maps = []
    for b in range(B):
        xkT = np.ascontiguousarray(np.asarray(key[b], f32).T.astype(_nbf16))
        xvT = np.ascontiguousarray(np.asarray(value[b], f32).T.astype(_nbf16))
        zT = np.ascontiguousarray(
            (1 - np.asarray(mask[b, 0])).T.astype(_nbf16)
        )  # [S2, S1_full]
        qT = np.ascontiguousarray(np.asarray(query[b], f32).T.astype(_nbf16))  # [D, S]
        for hf in range(2):
            sl = slice(hf * S1, (hf + 1) * S1)
            in_maps.append(
                dict(
                    xqT=np.ascontiguousarray(qT[:, sl]),
                    xkT=xkT,
                    xvT=xvT,
                    zmT=np.ascontiguousarray(zT[:, sl]),
                    wq=wq16, wk=wk16, wv=wv16, wo=wo16,
                    bq=bq_r, bk=bk_r, bv=bv_r, bo=bo_r,
                )
            )
    return in_maps


def kernel(value, key, query, mask, Wq, bq, Wk, bk, Wv, bv, Wo, bo, **run_kwargs):
    nc = _get_nc()
    in_maps = _prep_inputs(
        value, key, query, mask, Wq, bq, Wk, bk, Wv, bv, Wo, bo
    )
    res = bass_utils.run_bass_kernel_spmd(
        nc, in_maps, core_ids=list(range(8)), **run_kwargs
    )
    out = np.empty((B, S, D), np.float32)
    for c in range(8):
        b, hf = c // 2, c % 2
        out[b, hf * S1 : (hf + 1) * S1, :] = res.results[c]["out"]
    if run_kwargs:
        _CACHE["last_results"] = res
    return out
